# revision 1
# baseline (speedup 1.0000x reference)
"""Trainium2 Bass kernel for a 2-layer GAT + global mean pool + linear head.

Strategy (8 NeuronCores, SPMD):
  - Nodes are sorted by in-degree (desc) and dealt round-robin across the 8
    cores, so every core gets an equal, degree-matched shard. Each core owns
    the destination side of its shard's edges (graph/data parallel).
  - Per 128-node tile, in-edges live in a padded ELL table [128, D_t] of
    int32 source indices (D_t = max degree in tile; degree sorting makes the
    padding ~10%). Pad slots point at a dedicated table row whose attention
    logit contribution is -1e30 (=> alpha 0).
  - Layer tables (projected features + fused attention terms) are bf16,
    replicated across cores with an AllGather, and per-edge rows are fetched
    with indirect DMA gathers (the "halo exchange" for cross-shard edges).
  - Attention softmax + weighted aggregation are dense [128, D_t] tensor ops
    (DVE/ACT); projections and pooling one-hot reduction run on TensorE.
  - Global mean pool partial sums are AllReduced; every core computes the
    tiny final linear; core 0's output is returned.

The att_src/att_dst reductions are folded into the projection weights
(x @ W then <., a> == x @ (W a)), so per-edge attention terms ride along in
the same gathered row as the features.
"""

import numpy as np
import ml_dtypes

NC = 8
NEG_SLOPE = 0.2
BF16 = ml_dtypes.bfloat16

_NEFF_CACHE = {}


# --------------------------------------------------------------------------
# Host-side preprocessing (indexing / sharding only; all FLOPs run on device)
# --------------------------------------------------------------------------

def _host_prep(x, edge_index, batch, W1, att_src1, att_dst1, bias1,
               W2, att_src2, att_dst2, bias2, lin_w, lin_b):
    x = np.ascontiguousarray(np.asarray(x, np.float32))
    ei = np.asarray(edge_index).astype(np.int64)
    batch = np.asarray(batch).astype(np.int64)
    N, F = x.shape
    NG = 64

    src = np.concatenate([ei[0], np.arange(N, dtype=np.int64)])
    dst = np.concatenate([ei[1], np.arange(N, dtype=np.int64)])
    deg = np.bincount(dst, minlength=N)

    order1 = np.argsort(-deg, kind="stable")
    rank1 = np.empty(N, np.int64)
    rank1[order1] = np.arange(N)
    lo_e1 = (rank1[src] % NC) < (NC // 2)
    cl1 = np.bincount(dst, weights=lo_e1.astype(np.float64), minlength=N)
    # two-level sort: degree desc, then lo-window in-edge count (keeps both
    # lo and hi ELL blocks tightly packed within each 128-node tile)
    order = np.lexsort((cl1, -deg))
    rank = np.empty(N, np.int64)
    rank[order] = np.arange(N)
    core_of = rank % NC
    pos_of = rank // NC

    SH = -(-N // (NC * 128)) * 128                   # shard rows (6272)
    NT = SH // 128
    NTBL = SH * NC
    PAD_IDX = SH - 1                                 # core 0's pad row

    G = core_of * SH + pos_of                        # global gather row id

    HALF = NTBL // 2
    lo_e = G[src] < HALF                              # final window per edge
    cl = np.bincount(dst, weights=lo_e.astype(np.float64),
                     minlength=N).astype(np.int64)
    chi = deg - cl

    # per-tile lo/hi widths: max over the 1024 ranks (8 cores) of each tile
    D_lo, D_hi = [], []
    cls, chs = cl[order], chi[order]
    for t in range(NT):
        r0 = NC * 128 * t
        r1 = min(r0 + NC * 128, N)
        if r0 < N:
            D_lo.append(int(cls[r0:r1].max()))
            D_hi.append(int(chs[r0:r1].max()))
        else:
            D_lo.append(0)
            D_hi.append(0)
    D = [a + b for a, b in zip(D_lo, D_hi)]
    off = np.zeros(NT + 1, np.int64)
    off[1:] = np.cumsum(D)
    S = int(off[NT])

    # ELL slot assignment: per dst, lo-window edges first, then hi-window
    key = rank[dst] * 2 + (1 - lo_e.astype(np.int64))
    eorder = np.argsort(key, kind="stable")
    rs = rank[dst][eorder]
    first = np.searchsorted(rs, rs, side="left")
    d_slot = np.arange(len(rs), dtype=np.int64) - first
    lo_s = lo_e[eorder]
    srcg = G[src[eorder]]
    wrow = np.where(lo_s, srcg, srcg - HALF).astype(np.int16)
    ek = (rs % NC).astype(np.int64)
    epos = rs // NC
    et = epos // 128
    ep = epos % 128
    D_lo_a = np.asarray(D_lo, np.int64)
    cl_d = cl[dst][eorder]
    ecol = off[et] + np.where(lo_s, d_slot, D_lo_a[et] + (d_slot - cl_d))

    idx_all = np.full((NC, 128, S), PAD_IDX, np.int16)
    idx_all[ek, ep, ecol] = wrow

    # gather chunk schedule: (tile, slot_off, nslots, window), <=8 slots each
    chunks = []
    for t in range(NT):
        for w, (base, dw) in enumerate([(0, D_lo[t]), (D_lo[t], D_hi[t])]):
            s0 = 0
            while s0 < dw:
                ns = min(8, dw - s0)
                chunks.append((t, base + s0, ns, w))
                s0 += ns
    S16 = sum(8 * ns for (_, _, ns, _) in chunks)

    # wrapped int16 index input: per chunk, i = slot*128 + p lives at
    # [16r + i%16, i//16] (replicated over the 8 Q7 core groups)
    idx16 = np.empty((NC, 128, S16), np.int16)
    cw = 0
    for (t, so, ns, w) in chunks:
        for k in range(NC):
            v = idx_all[k, :, off[t] + so: off[t] + so + ns]  # [128, ns]
            flat = v.T.reshape(-1)                            # i = slot*128+p
            wr = flat.reshape(8 * ns, 16).T                   # [16, 8*ns]
            idx16[k, :, cw:cw + 8 * ns] = np.tile(wr, (8, 1))
        cw += 8 * ns
    assert cw == S16

    # pooling one-hot (per core, per tile) and counts
    B = np.zeros((NC, SH, NG), np.float32)
    B[core_of, pos_of, batch] = 1.0
    B = B.reshape(NC, NT, 128, NG).astype(BF16)
    counts = np.bincount(batch, minlength=NG).astype(np.float32)
    inv_counts = (1.0 / np.maximum(counts, 1.0)).reshape(NG, 1)

    # per-core x, transposed for TensorE (feature-major)
    xs = np.zeros((NC, SH, F), np.float32)
    xs[core_of, pos_of] = x
    xt = np.ascontiguousarray(xs.transpose(0, 2, 1))  # [NC, F, SH]

    # fold attention vectors into projection weights
    W1 = np.asarray(W1, np.float32)
    W2 = np.asarray(W2, np.float32)
    a_s1 = np.asarray(att_src1, np.float32)           # [8, 8]
    a_d1 = np.asarray(att_dst1, np.float32)
    a_s2 = np.asarray(att_src2, np.float32)           # [1, 32]
    a_d2 = np.asarray(att_dst2, np.float32)
    W1r = W1.reshape(F, 8, 8)
    w_as1 = np.einsum("khc,hc->kh", W1r, a_s1)
    w_ad1 = np.einsum("khc,hc->kh", W1r, a_d1)
    W1p = np.concatenate([W1, w_as1, w_ad1], axis=1)  # [512, 80]
    W2p = np.concatenate([W2, W2 @ a_s2[0][:, None], W2 @ a_d2[0][:, None]],
                         axis=1)                      # [64, 34]

    pad1 = np.zeros((1, 128), np.float32)
    pad1[0, 64:72] = -1e30
    pad2 = np.zeros((1, 128), np.float32)
    pad2[0, 32] = -1e30

    common = {
        "w1p": np.ascontiguousarray(W1p),
        "w2p": np.ascontiguousarray(W2p),
        "bias1r": np.broadcast_to(np.asarray(bias1, np.float32), (128, 64)).copy(),
        "bias2r": np.broadcast_to(np.asarray(bias2, np.float32), (128, 32)).copy(),
        "linw": np.asarray(lin_w, np.float32).reshape(32, 2).copy(),
        "linbr": np.broadcast_to(np.asarray(lin_b, np.float32), (128, 2)).copy(),
        "invc": inv_counts,
        "pad1": pad1.astype(BF16),
        "pad2": pad2.astype(BF16),
    }
    in_maps = []
    for k in range(NC):
        m = dict(common)
        m["xt"] = xt[k]
        m["idx"] = np.ascontiguousarray(idx16[k])
        m["bmat"] = np.ascontiguousarray(B[k])
        in_maps.append(m)

    meta = (N, F, SH, NT, NTBL, S16, tuple(D), NG, tuple(chunks))
    return in_maps, meta


# --------------------------------------------------------------------------
# Bass kernel builder
# --------------------------------------------------------------------------

def _build(meta):
    import concourse.bass as bass
    import concourse.bacc as bacc
    import concourse.tile as tile
    import concourse.mybir as mybir
    from concourse.masks import make_identity

    N, F, SH, NT, NTBL, S16, D, NG, chunks = meta
    from concourse import library_config
    HALF = NTBL // 2
    # wrapped-idx column offset per chunk
    chunk_cols = []
    cw = 0
    for (_, _, ns, _) in chunks:
        chunk_cols.append(cw)
        cw += 8 * ns
    tile_chunks = {t: [] for t in range(NT)}
    for ci, (t, so, ns, w) in enumerate(chunks):
        tile_chunks[t].append((so, ns, w, chunk_cols[ci]))
    FK = F // 128                                     # k-chunks in proj1
    f32 = mybir.dt.float32
    bf16 = mybir.dt.bfloat16
    AT = mybir.AluOpType
    AF = mybir.ActivationFunctionType
    AX = mybir.AxisListType

    nc = bacc.Bacc("TRN2", target_bir_lowering=False, debug=False,
                   num_devices=NC)

    xt_d = nc.dram_tensor("xt", [F, SH], f32, kind="ExternalInput")
    idx_d = nc.dram_tensor("idx", [128, S16], mybir.dt.int16, kind="ExternalInput")
    bmat_d = nc.dram_tensor("bmat", [NT, 128, NG], bf16, kind="ExternalInput")
    w1p_d = nc.dram_tensor("w1p", [F, 80], f32, kind="ExternalInput")
    w2p_d = nc.dram_tensor("w2p", [64, 34], f32, kind="ExternalInput")
    b1_d = nc.dram_tensor("bias1r", [128, 64], f32, kind="ExternalInput")
    b2_d = nc.dram_tensor("bias2r", [128, 32], f32, kind="ExternalInput")
    linw_d = nc.dram_tensor("linw", [32, 2], f32, kind="ExternalInput")
    linb_d = nc.dram_tensor("linbr", [128, 2], f32, kind="ExternalInput")
    invc_d = nc.dram_tensor("invc", [NG, 1], f32, kind="ExternalInput")
    pad1_d = nc.dram_tensor("pad1", [1, 128], bf16, kind="ExternalInput")
    pad2_d = nc.dram_tensor("pad2", [1, 128], bf16, kind="ExternalInput")
    out_d = nc.dram_tensor("out", [NG, 2], f32, kind="ExternalOutput")

    rg = [list(range(NC))]

    with tile.TileContext(nc) as tc:
        nc.gpsimd.load_library(library_config.mlp)
        with (
            tc.tile_pool(name="const", bufs=1) as cp,
            tc.tile_pool(name="dram", bufs=1, space="DRAM") as dp,
        ):
            # ---------------- constants ----------------
            identity = cp.tile([128, 128], f32, name="identity")
            make_identity(nc, identity[:])

            w1bf = []
            for kk in range(FK):
                wf = cp.tile([128, 80], f32, name=f"w1f{kk}")
                nc.sync.dma_start(wf[:], w1p_d.ap()[128 * kk:128 * (kk + 1), :])
                wb = cp.tile([128, 80], bf16, name=f"w1b{kk}")
                nc.vector.tensor_copy(wb[:], wf[:])
                w1bf.append(wb)
            w2f = cp.tile([64, 34], f32, name="w2f")
            nc.sync.dma_start(w2f[:], w2p_d.ap()[:])
            w2bf = cp.tile([64, 34], bf16, name="w2bf")
            nc.vector.tensor_copy(w2bf[:], w2f[:])

            bias1 = cp.tile([128, 64], f32, name="bias1")
            nc.sync.dma_start(bias1[:], b1_d.ap()[:])
            bias2 = cp.tile([128, 32], f32, name="bias2")
            nc.sync.dma_start(bias2[:], b2_d.ap()[:])
            linw = cp.tile([32, 2], f32, name="linw_t")
            nc.sync.dma_start(linw[:], linw_d.ap()[:])
            linb = cp.tile([128, 2], f32, name="linb_t")
            nc.sync.dma_start(linb[:], linb_d.ap()[:])
            invc = cp.tile([NG, 1], f32, name="invc_t")
            nc.sync.dma_start(invc[:], invc_d.ap()[:])

            ad1_all = cp.tile([128, NT * 8], f32, name="ad1_all")
            ad2_all = cp.tile([128, NT], f32, name="ad2_all")
            idx_sb = cp.tile([128, S16], mybir.dt.int16, name="idx_sb")
            nc.sync.dma_start(idx_sb[:], idx_d.ap()[:])

            # ---------------- DRAM tables ----------------
            shard1 = dp.tile([SH, 128], bf16, name="shard1")
            table1 = dp.tile([NTBL, 128], bf16, name="table1", addr_space="Shared")
            shard2 = dp.tile([SH, 128], bf16, name="shard2")
            table2 = dp.tile([NTBL, 128], bf16, name="table2", addr_space="Shared")
            pool_in = dp.tile([NG, 32], f32, name="pool_in")
            pool_out = dp.tile([NG, 32], f32, name="pool_out", addr_space="Shared")

            # ---------------- phase A: proj1 -> shard1 ----------------
            with (
                tc.tile_pool(name="pa", bufs=2) as pa,
                tc.tile_pool(name="pax", bufs=1) as pax,
                tc.tile_pool(name="pap", bufs=4, space="PSUM") as pap,
            ):
                xball = []
                for kk in range(FK):
                    xf = pa.tile([128, SH], f32, tag="xf")
                    nc.sync.dma_start(xf[:],
                                      xt_d.ap()[128 * kk:128 * (kk + 1), :])
                    xb = pax.tile([128, SH], bf16, name=f"xball{kk}")
                    nc.vector.tensor_copy(xb[:], xf[:])
                    xball.append(xb)
                for t in range(NT):
                    ps = pap.tile([128, 80], f32, tag="proj1")
                    for kk in range(FK):
                        nc.tensor.matmul(
                            out=ps[:],
                            lhsT=xball[kk][:, 128 * t:128 * (t + 1)],
                            rhs=w1bf[kk][:],
                            start=(kk == 0), stop=(kk == FK - 1))
                    hb = pa.tile([128, 128], bf16, tag="hb")
                    nc.vector.tensor_copy(hb[:, 0:72], ps[:, 0:72])
                    nc.vector.tensor_copy(ad1_all[:, 8 * t:8 * (t + 1)],
                                          ps[:, 72:80])
                    nc.sync.dma_start(shard1[128 * t:128 * (t + 1), :], hb[:])
            nc.sync.dma_start(shard1[SH - 1:SH, :], pad1_d.ap()[:])

            nc.gpsimd.collective_compute(
                "AllGather", AT.bypass, replica_groups=rg,
                ins=[shard1.opt()], outs=[table1.opt()])

            # ---------------- phase C: L1 edges + proj2 -> shard2 ----------
            with (
                tc.tile_pool(name="pc", bufs=3) as pc,
                tc.tile_pool(name="pcs", bufs=4) as pcs,
                tc.tile_pool(name="pcp", bufs=4, space="PSUM") as pcp,
            ):
                for t in range(NT):
                    Dt = D[t]
                    g1 = pc.tile([128, Dt, 128], bf16, tag="g1")
                    for (so, ns, w, ccol) in tile_chunks[t]:
                        nidx = 128 * ns
                        nc.gpsimd.dma_gather(
                            g1[:, so:so + ns, :],
                            table1[0:HALF] if w == 0 else table1[HALF:NTBL],
                            idx_sb[:, ccol:ccol + 8 * ns],
                            nidx, nidx, 128)

                    # logits z[p, h, d] = as1[src] + ad1[dst]
                    z = pcs.tile([128, Dt, 8], f32, tag="z")
                    nc.vector.tensor_tensor(
                        out=z[:],
                        in0=g1[:, :, 64:72],
                        in1=ad1_all[:, 8 * t:8 * (t + 1)].unsqueeze(1)
                            .broadcast_to([128, Dt, 8]),
                        op=AT.add)
                    # leaky relu (slope 0.2) = max(z, 0.2 z)
                    za = pcs.tile([128, Dt, 8], f32, tag="za")
                    nc.vector.scalar_tensor_tensor(
                        out=za[:], in0=z[:], scalar=NEG_SLOPE, in1=z[:],
                        op0=AT.mult, op1=AT.max)
                    p1 = pcs.tile([128, Dt, 8], bf16, tag="p1")
                    nc.scalar.activation(
                        p1[:].rearrange("p d h -> p (d h)"),
                        za[:].rearrange("p d h -> p (d h)"), AF.Exp)
                    den = pcs.tile([128, 8], f32, tag="den")
                    nc.vector.tensor_reduce(
                        den[:], p1[:].rearrange("p d h -> p h d"),
                        axis=AX.X, op=AT.add)
                    nc.vector.tensor_scalar_add(den[:], den[:], 1e-16)
                    rec = pcs.tile([128, 8], f32, tag="rec")
                    nc.vector.reciprocal(rec[:], den[:])

                    # weighted sum over slots
                    prod = pc.tile([128, Dt, 64], bf16, tag="prod")
                    nc.vector.tensor_tensor(
                        out=prod[:].rearrange("p d (h c) -> p d h c", h=8),
                        in0=g1[:, :, 0:64].rearrange("p d (h c) -> p d h c", h=8),
                        in1=p1[:].unsqueeze(3)
                            .broadcast_to([128, Dt, 8, 8]),
                        op=AT.mult)
                    o1 = pcs.tile([128, 64], f32, tag="o1")
                    nc.vector.tensor_reduce(
                        o1[:], prod[:].rearrange("p d c -> p c d"),
                        axis=AX.X, op=AT.add)
                    nc.vector.tensor_tensor(
                        out=o1[:].rearrange("p (h c) -> p h c", h=8),
                        in0=o1[:].rearrange("p (h c) -> p h c", h=8),
                        in1=rec[:].unsqueeze(2).broadcast_to([128, 8, 8]),
                        op=AT.mult)
                    nc.vector.tensor_add(o1[:], o1[:], bias1[:])

                    # ELU: max(y,0) + exp(min(y,0)) - 1
                    mn = pcs.tile([128, 64], f32, tag="mn")
                    nc.vector.tensor_scalar_min(mn[:], o1[:], 0.0)
                    ex = pcs.tile([128, 64], f32, tag="ex")
                    nc.scalar.activation(ex[:], mn[:], AF.Exp)
                    x2 = pcs.tile([128, 64], f32, tag="x2")
                    nc.vector.scalar_tensor_tensor(
                        out=x2[:], in0=o1[:], scalar=0.0, in1=ex[:],
                        op0=AT.max, op1=AT.add)
                    nc.vector.tensor_scalar_add(x2[:], x2[:], -1.0)

                    # proj2
                    tp = pcp.tile([64, 128], f32, tag="tp", space="PSUM")
                    nc.tensor.transpose(out=tp[:], in_=x2[:], identity=identity[:])
                    x2T = pcs.tile([64, 128], bf16, tag="x2T")
                    nc.vector.tensor_copy(x2T[:], tp[:])
                    ps2 = pcp.tile([128, 34], f32, tag="ps2", space="PSUM")
                    nc.tensor.matmul(out=ps2[:], lhsT=x2T[:], rhs=w2bf[:],
                                     start=True, stop=True)
                    hb2 = pcs.tile([128, 128], bf16, tag="hb2")
                    nc.vector.tensor_copy(hb2[:, 0:33], ps2[:, 0:33])
                    nc.vector.tensor_copy(ad2_all[:, t:t + 1], ps2[:, 33:34])
                    nc.sync.dma_start(shard2[128 * t:128 * (t + 1), :], hb2[:])
            nc.sync.dma_start(shard2[SH - 1:SH, :], pad2_d.ap()[:])

            nc.gpsimd.collective_compute(
                "AllGather", AT.bypass, replica_groups=rg,
                ins=[shard2.opt()], outs=[table2.opt()])

            # ---------------- phase E: L2 edges + pooling ----------------
            with (
                tc.tile_pool(name="pe", bufs=3) as pe,
                tc.tile_pool(name="pes", bufs=4) as pes,
                tc.tile_pool(name="pep", bufs=1, space="PSUM") as pep,
            ):
                pool_ps = pep.tile([NG, 32], f32, name="pool_ps", space="PSUM")
                for t in range(NT):
                    Dt = D[t]
                    g2 = pe.tile([128, Dt, 128], bf16, tag="g2")
                    for (so, ns, w, ccol) in tile_chunks[t]:
                        nidx = 128 * ns
                        nc.gpsimd.dma_gather(
                            g2[:, so:so + ns, :],
                            table2[0:HALF] if w == 0 else table2[HALF:NTBL],
                            idx_sb[:, ccol:ccol + 8 * ns],
                            nidx, nidx, 128)

                    z2 = pes.tile([128, Dt], f32, tag="z2")
                    nc.vector.tensor_tensor(
                        out=z2[:], in0=g2[:, :, 32],
                        in1=ad2_all[:, t:t + 1].to_broadcast([128, Dt]),
                        op=AT.add)
                    za2 = pes.tile([128, Dt], f32, tag="za2")
                    nc.vector.scalar_tensor_tensor(
                        out=za2[:], in0=z2[:], scalar=NEG_SLOPE, in1=z2[:],
                        op0=AT.mult, op1=AT.max)
                    p2 = pes.tile([128, Dt], bf16, tag="p2")
                    nc.scalar.activation(p2[:], za2[:], AF.Exp)
                    den2 = pes.tile([128, 1], f32, tag="den2")
                    nc.vector.tensor_reduce(den2[:], p2[:], axis=AX.X, op=AT.add)
                    nc.vector.tensor_scalar_add(den2[:], den2[:], 1e-16)
                    rec2 = pes.tile([128, 1], f32, tag="rec2")
                    nc.vector.reciprocal(rec2[:], den2[:])

                    prod2 = pe.tile([128, Dt, 32], bf16, tag="prod2")
                    nc.vector.tensor_tensor(
                        out=prod2[:],
                        in0=g2[:, :, 0:32],
                        in1=p2[:].unsqueeze(2).broadcast_to([128, Dt, 32]),
                        op=AT.mult)
                    o2 = pes.tile([128, 32], f32, tag="o2")
                    nc.vector.tensor_reduce(
                        o2[:], prod2[:].rearrange("p d c -> p c d"),
                        axis=AX.X, op=AT.add)
                    nc.vector.tensor_scalar(out=o2[:], in0=o2[:],
                                            scalar1=rec2[:], scalar2=None,
                                            op0=AT.mult)
                    nc.vector.tensor_add(o2[:], o2[:], bias2[:])
                    h2b = pes.tile([128, 32], bf16, tag="h2b")
                    nc.vector.tensor_copy(h2b[:], o2[:])

                    bt = pes.tile([128, NG], bf16, tag="bt")
                    nc.sync.dma_start(bt[:], bmat_d.ap()[t])
                    nc.tensor.matmul(out=pool_ps[:], lhsT=bt[:], rhs=h2b[:],
                                     start=(t == 0), stop=(t == NT - 1))

                # ---------------- phase F: pool + head ----------------
                pooled = pes.tile([NG, 32], f32, name="pooled")
                nc.vector.tensor_copy(pooled[:], pool_ps[:])
                nc.sync.dma_start(pool_in[:], pooled[:])
                nc.gpsimd.collective_compute(
                    "AllReduce", AT.add, replica_groups=rg,
                    ins=[pool_in.opt()], outs=[pool_out.opt()])
                pooled2 = pes.tile([NG, 32], f32, name="pooled2")
                nc.sync.dma_start(pooled2[:], pool_out[:])
                nc.vector.tensor_scalar(out=pooled2[:], in0=pooled2[:],
                                        scalar1=invc[:], scalar2=None,
                                        op0=AT.mult)
                tpf = pep.tile([32, NG], f32, name="tpf", space="PSUM")
                nc.tensor.transpose(out=tpf[:], in_=pooled2[:],
                                    identity=identity[:NG, :NG])
                pooledT = pes.tile([32, NG], f32, name="pooledT")
                nc.vector.tensor_copy(pooledT[:], tpf[:])
                fin = pep.tile([NG, 2], f32, name="fin", space="PSUM")
                nc.tensor.matmul(out=fin[:], lhsT=pooledT[:], rhs=linw[:],
                                 start=True, stop=True)
                res = pes.tile([NG, 2], f32, name="res")
                nc.vector.tensor_tensor(out=res[:], in0=fin[:],
                                        in1=linb[:NG, :], op=AT.add)
                nc.sync.dma_start(out_d.ap()[:], res[:])

    nc.compile()
    return nc


# --------------------------------------------------------------------------
# Entry point
# --------------------------------------------------------------------------

def kernel(**inputs):
    from concourse.bass_utils import run_bass_kernel_spmd

    in_maps, meta = _host_prep(**inputs)
    nc = _NEFF_CACHE.get(meta)
    if nc is None:
        nc = _build(meta)
        _NEFF_CACHE[meta] = nc
    res = run_bass_kernel_spmd(nc, in_maps, core_ids=list(range(NC)))
    return np.asarray(res.results[0]["out"], np.float32)



# revision 6
# speedup vs baseline: 3.2586x; 3.2586x over previous
"""Trainium2 Bass kernel for a 2-layer GAT + global mean pool + linear head.

Strategy (8 NeuronCores, SPMD):
  - Nodes sorted by in-degree (desc) and dealt round-robin across the 8
    cores; each core owns the destination side of its shard's edges.
  - Per 128-node tile, in-edges live in a padded ELL table [128, D_t] of
    source indices (D_t = max degree in tile; degree sorting keeps padding
    ~2%). Layer tables (projected features + fused attention logits) are
    bf16, replicated across cores with an AllGather.
  - Edge rows are fetched with gpsimd dma_gather using 512-byte tokens,
    each covering a PAIR of table rows (idx = src_row >> 1 fits int16 for
    the full 50176-row table, so no lo/hi window split and no binomial
    padding blowup). A parity-mask predicated copy selects the right half.
    Gathers are issued round-robin on all 4 SWDGE queues (each queue is a
    separate Q7 core pair, ~3.3x descriptor-generation throughput).
  - Pad slots of real rows point at a pair whose attention logit is -1e30
    (alpha 0); slots of node-less tail rows point at an all-zero pair so
    the softmax denominator stays finite without epsilon ops.
  - Attention softmax + weighted aggregation are dense [128, D_t] ops
    split across DVE and ACT; projections + pooling one-hot run on TensorE.
  - Global mean pool partials are AllReduced; every core computes the tiny
    final head; core 0's output is returned.
"""

import numpy as np
import ml_dtypes

NC = 8
NEG_SLOPE = 0.2
BF16 = ml_dtypes.bfloat16

_NEFF_CACHE = {}


# --------------------------------------------------------------------------
# Host-side preprocessing (indexing / sharding only; all FLOPs on device)
# --------------------------------------------------------------------------

def _host_prep(x, edge_index, batch, W1, att_src1, att_dst1, bias1,
               W2, att_src2, att_dst2, bias2, lin_w, lin_b):
    x = np.ascontiguousarray(np.asarray(x, np.float32))
    ei = np.asarray(edge_index).astype(np.int64)
    batch = np.asarray(batch).astype(np.int64)
    N, F = x.shape
    NG = 64

    src = np.concatenate([ei[0], np.arange(N, dtype=np.int64)])
    dst = np.concatenate([ei[1], np.arange(N, dtype=np.int64)])
    deg = np.bincount(dst, minlength=N)

    order = np.argsort(-deg, kind="stable")
    rank = np.empty(N, np.int64)
    rank[order] = np.arange(N)
    core_of = rank % NC
    pos_of = rank // NC

    SH = -(-N // (NC * 128)) * 128                   # shard rows per core
    NT = SH // 128
    NTBL = SH * NC
    NPAIR = NTBL // 2
    # max positions actually used per core
    max_pos = int(pos_of.max())
    assert max_pos + 4 <= SH, "need >=4 spare rows for pad pairs"
    # zero pair (as=0 -> alpha contribution exp(0), h=0) for node-less rows
    zp_pos = SH - 4                                   # core 0 rows zp, zp+1
    ZPAD = zp_pos // 2                                # pair idx (core 0)
    np_pos = SH - 2                                   # -1e30 pair rows
    NPAD = np_pos // 2
    assert zp_pos % 2 == 0 and np_pos % 2 == 0

    G = core_of * SH + pos_of                        # global table row id

    ds = deg[order]
    D = []
    for t in range(NT):
        r0 = NC * 128 * t
        D.append(int(ds[r0]) if r0 < N else 1)       # sorted desc -> max
    D = [max(d, 1) for d in D]
    off = np.zeros(NT + 1, np.int64)
    off[1:] = np.cumsum(D)
    S = int(off[NT])

    # ELL slot assignment (slot = per-dst running index)
    eorder = np.argsort(rank[dst], kind="stable")
    rs = rank[dst][eorder]
    first = np.searchsorted(rs, rs, side="left")
    d_slot = np.arange(len(rs), dtype=np.int64) - first
    srcg = G[src[eorder]]
    wrow = (srcg >> 1).astype(np.int16)
    par = (srcg & 1).astype(np.float32)
    ek = (rs % NC).astype(np.int64)
    epos = rs // NC
    et = epos // 128
    ep = epos % 128
    ecol = off[et] + d_slot

    idx_all = np.full((NC, 128, S), NPAD, np.int16)
    pm_all = np.zeros((NC, 128, S), np.float32)
    idx_all[ek, ep, ecol] = wrow
    pm_all[ek, ep, ecol] = par
    # node-less tail rows -> zero pair (keeps softmax denominator finite)
    for t in range(NT):
        lo = NC * 128 * t
        hi = min(lo + NC * 128, 10**18)
        if lo + NC * 128 > N:                         # tile has unused ranks
            for k in range(NC):
                # positions pos in tile t with rank = pos*NC + k >= N
                p0 = max(0, -(-(N - k) // NC) - t * 128)   # first unused p
                if p0 < 128:
                    idx_all[k, p0:, off[t]:off[t + 1]] = ZPAD
                    pm_all[k, p0:, off[t]:off[t + 1]] = 0.0

    # gather chunk schedule: (tile, slot_off, nslots), <=8 slots (1024 idxs)
    chunks = []
    for t in range(NT):
        s0 = 0
        while s0 < D[t]:
            ns = min(8, D[t] - s0)
            chunks.append((t, s0, ns))
            s0 += ns
    S16 = sum(8 * ns for (_, _, ns) in chunks)

    # wrapped int16 index input: per chunk, i = slot*128 + p lives at
    # [16r + i%16, i//16] (replicated over the 8 Q7 core groups)
    idx16 = np.empty((NC, 128, S16), np.int16)
    cw = 0
    for (t, so, ns) in chunks:
        for k in range(NC):
            v = idx_all[k, :, off[t] + so: off[t] + so + ns]  # [128, ns]
            flat = v.T.reshape(-1)                            # i = slot*128+p
            wr = flat.reshape(8 * ns, 16).T                   # [16, 8*ns]
            idx16[k, :, cw:cw + 8 * ns] = np.tile(wr, (8, 1))
        cw += 8 * ns
    assert cw == S16

    # pooling one-hot (per core, per tile) and counts
    B = np.zeros((NC, SH, NG), np.float32)
    B[core_of, pos_of, batch] = 1.0
    B = B.reshape(NC, NT, 128, NG).astype(BF16)
    counts = np.bincount(batch, minlength=NG).astype(np.float32)
    inv_counts = (1.0 / np.maximum(counts, 1.0)).reshape(NG, 1)

    # per-core x, transposed for TensorE (feature-major), bf16 on host
    xs = np.zeros((NC, SH, F), np.float32)
    xs[core_of, pos_of] = x
    xt = np.ascontiguousarray(xs.transpose(0, 2, 1)).astype(BF16)

    # fold attention vectors into projection weights
    W1 = np.asarray(W1, np.float32)
    W2 = np.asarray(W2, np.float32)
    a_s1 = np.asarray(att_src1, np.float32)           # [8, 8]
    a_d1 = np.asarray(att_dst1, np.float32)
    a_s2 = np.asarray(att_src2, np.float32)           # [1, 32]
    a_d2 = np.asarray(att_dst2, np.float32)
    W1r = W1.reshape(F, 8, 8)
    w_as1 = np.einsum("khc,hc->kh", W1r, a_s1)
    w_ad1 = np.einsum("khc,hc->kh", W1r, a_d1)
    W1p = np.concatenate([W1, w_as1, w_ad1], axis=1)  # [512, 80]
    W2p = np.concatenate([W2, W2 @ a_s2[0][:, None], W2 @ a_d2[0][:, None]],
                         axis=1)                      # [64, 34]

    pad1 = np.zeros((2, 128), np.float32)
    pad1[:, 64:72] = -1e30
    pad2 = np.zeros((2, 128), np.float32)
    pad2[:, 32] = -1e30

    common = {
        "w1p": np.ascontiguousarray(W1p),
        "w2p": np.ascontiguousarray(W2p),
        "bias1r": np.broadcast_to(np.asarray(bias1, np.float32), (128, 64)).copy(),
        "bias2r": np.broadcast_to(np.asarray(bias2, np.float32), (128, 32)).copy(),
        "linw": np.asarray(lin_w, np.float32).reshape(32, 2).copy(),
        "linbr": np.broadcast_to(np.asarray(lin_b, np.float32), (128, 2)).copy(),
        "invc": inv_counts,
        "pad1": pad1.astype(BF16),
        "pad2": pad2.astype(BF16),
    }
    in_maps = []
    for k in range(NC):
        m = dict(common)
        m["xt"] = xt[k]
        m["idx"] = np.ascontiguousarray(idx16[k])
        m["pm"] = np.ascontiguousarray(pm_all[k].astype(np.uint8))
        m["bmat"] = np.ascontiguousarray(B[k])
        in_maps.append(m)

    meta = (N, F, SH, NT, NTBL, S16, tuple(D), NG, tuple(chunks),
            zp_pos, np_pos)
    return in_maps, meta


# --------------------------------------------------------------------------
# Bass kernel builder
# --------------------------------------------------------------------------

def _build(meta):
    import concourse.bass as bass
    import concourse.bacc as bacc
    import concourse.tile as tile
    import concourse.mybir as mybir
    from concourse.masks import make_identity
    from concourse import library_config

    (N, F, SH, NT, NTBL, S16, D, NG, chunks, zp_pos, np_pos) = meta
    NPAIR = NTBL // 2
    off = [0]
    for d in D:
        off.append(off[-1] + d)
    chunk_cols = []
    cw = 0
    for (_, _, ns) in chunks:
        chunk_cols.append(cw)
        cw += 8 * ns
    tile_chunks = {t: [] for t in range(NT)}
    for ci, (t, so, ns) in enumerate(chunks):
        tile_chunks[t].append((so, ns, chunk_cols[ci]))
    FK = F // 128
    f32 = mybir.dt.float32
    bf16 = mybir.dt.bfloat16
    AT = mybir.AluOpType
    AF = mybir.ActivationFunctionType
    AX = mybir.AxisListType

    nc = bacc.Bacc("TRN2", target_bir_lowering=False, debug=False,
                   num_devices=NC, num_swdge_queues=4)

    xt_d = nc.dram_tensor("xt", [F, SH], bf16, kind="ExternalInput")
    idx_d = nc.dram_tensor("idx", [128, S16], mybir.dt.int16, kind="ExternalInput")
    pm_d = nc.dram_tensor("pm", [128, off[-1]], mybir.dt.uint8, kind="ExternalInput")
    bmat_d = nc.dram_tensor("bmat", [NT, 128, NG], bf16, kind="ExternalInput")
    w1p_d = nc.dram_tensor("w1p", [F, 80], f32, kind="ExternalInput")
    w2p_d = nc.dram_tensor("w2p", [64, 34], f32, kind="ExternalInput")
    b1_d = nc.dram_tensor("bias1r", [128, 64], f32, kind="ExternalInput")
    b2_d = nc.dram_tensor("bias2r", [128, 32], f32, kind="ExternalInput")
    linw_d = nc.dram_tensor("linw", [32, 2], f32, kind="ExternalInput")
    linb_d = nc.dram_tensor("linbr", [128, 2], f32, kind="ExternalInput")
    invc_d = nc.dram_tensor("invc", [NG, 1], f32, kind="ExternalInput")
    pad1_d = nc.dram_tensor("pad1", [2, 128], bf16, kind="ExternalInput")
    pad2_d = nc.dram_tensor("pad2", [2, 128], bf16, kind="ExternalInput")
    out_d = nc.dram_tensor("out", [NG, 2], f32, kind="ExternalOutput")

    rg = [list(range(NC))]
    gathers = []          # (BassInstruction, queue) for post-compile check
    qctr = [0]

    def gq():
        q = qctr[0] % 4
        qctr[0] += 1
        return q

    with tile.TileContext(nc) as tc:
        nc.gpsimd.load_library(library_config.mlp)
        with (
            tc.tile_pool(name="const", bufs=1) as cp,
            tc.tile_pool(name="dram", bufs=1, space="DRAM") as dp,
        ):
            # ---------------- constants ----------------
            identity = cp.tile([128, 128], f32, name="identity")
            make_identity(nc, identity[:])

            w1bf = []
            for kk in range(FK):
                wf = cp.tile([128, 80], f32, name=f"w1f{kk}")
                nc.sync.dma_start(wf[:], w1p_d.ap()[128 * kk:128 * (kk + 1), :])
                wb = cp.tile([128, 80], bf16, name=f"w1b{kk}")
                nc.vector.tensor_copy(wb[:], wf[:])
                w1bf.append(wb)
            w2f = cp.tile([64, 34], f32, name="w2f")
            nc.sync.dma_start(w2f[:], w2p_d.ap()[:])
            w2bf = cp.tile([64, 34], bf16, name="w2bf")
            nc.vector.tensor_copy(w2bf[:], w2f[:])

            bias1 = cp.tile([128, 64], f32, name="bias1")
            nc.sync.dma_start(bias1[:], b1_d.ap()[:])
            bias2 = cp.tile([128, 32], f32, name="bias2")
            nc.sync.dma_start(bias2[:], b2_d.ap()[:])
            linw = cp.tile([32, 2], f32, name="linw_t")
            nc.sync.dma_start(linw[:], linw_d.ap()[:])
            linb = cp.tile([128, 2], f32, name="linb_t")
            nc.sync.dma_start(linb[:], linb_d.ap()[:])
            invc = cp.tile([NG, 1], f32, name="invc_t")
            nc.sync.dma_start(invc[:], invc_d.ap()[:])

            ad1_all = cp.tile([128, NT * 8], f32, name="ad1_all")
            ad2_all = cp.tile([128, NT], f32, name="ad2_all")
            idx_sb = cp.tile([128, S16], mybir.dt.int16, name="idx_sb")
            nc.sync.dma_start(idx_sb[:], idx_d.ap()[:])
            pm_sb = cp.tile([128, off[-1]], mybir.dt.uint8, name="pm_sb")
            nc.sync.dma_start(pm_sb[:], pm_d.ap()[:])

            # ---------------- DRAM tables ----------------
            shard1 = dp.tile([SH, 128], bf16, name="shard1")
            table1 = dp.tile([NTBL, 128], bf16, name="table1", addr_space="Shared")
            shard2 = dp.tile([SH, 128], bf16, name="shard2")
            table2 = dp.tile([NTBL, 128], bf16, name="table2", addr_space="Shared")
            pool_in = dp.tile([NG, 32], f32, name="pool_in")
            pool_out = dp.tile([NG, 32], f32, name="pool_out", addr_space="Shared")

            t1pair = table1[:].rearrange("(r a) c -> r (a c)", a=2)
            t2pair = table2[:].rearrange("(r a) c -> r (a c)", a=2)

            # ---------------- phase A: proj1 -> shard1 ----------------
            with (
                tc.tile_pool(name="pa", bufs=2) as pa,
                tc.tile_pool(name="pax", bufs=1) as pax,
                tc.tile_pool(name="pap", bufs=4, space="PSUM") as pap,
            ):
                xball = []
                for kk in range(FK):
                    xb = pax.tile([128, SH], bf16, name=f"xball{kk}")
                    nc.sync.dma_start(xb[:],
                                      xt_d.ap()[128 * kk:128 * (kk + 1), :])
                    xball.append(xb)
                for t in range(NT):
                    ps = pap.tile([128, 80], f32, tag="proj1")
                    for kk in range(FK):
                        nc.tensor.matmul(
                            out=ps[:],
                            lhsT=xball[kk][:, 128 * t:128 * (t + 1)],
                            rhs=w1bf[kk][:],
                            start=(kk == 0), stop=(kk == FK - 1))
                    hb = pa.tile([128, 128], bf16, tag="hb")
                    nc.scalar.activation(hb[:, 0:72], ps[:, 0:72], AF.Copy)
                    nc.vector.tensor_copy(ad1_all[:, 8 * t:8 * (t + 1)],
                                          ps[:, 72:80])
                    nc.sync.dma_start(shard1[128 * t:128 * (t + 1), :], hb[:])
            nc.sync.dma_start(shard1[np_pos:np_pos + 2, :], pad1_d.ap()[:])

            nc.gpsimd.collective_compute(
                "AllGather", AT.bypass, replica_groups=rg,
                ins=[shard1.opt()], outs=[table1.opt()])

            # ---------------- phase C: L1 edges + proj2 -> shard2 ----------
            with (
                tc.tile_pool(name="pc", bufs=2) as pc,
                tc.tile_pool(name="pcs", bufs=3) as pcs,
                tc.tile_pool(name="pcp", bufs=4, space="PSUM") as pcp,
            ):
                for t in range(NT):
                    Dt = D[t]
                    g1 = pc.tile([128, Dt, 256], bf16, tag="g1")
                    for (so, ns, ccol) in tile_chunks[t]:
                        nidx = 128 * ns
                        gi = nc.gpsimd.dma_gather(
                            g1[:, so:so + ns, :], t1pair,
                            idx_sb[:, ccol:ccol + 8 * ns],
                            nidx, nidx, 256, queue_num=gq())
                        gathers.append(gi)
                    # parity select: g1s = par ? odd_half : even_half (72 cols)
                    pmv = pm_sb[:, off[t]:off[t] + Dt].unsqueeze(2) \
                        .broadcast_to([128, Dt, 72])
                    g1s = pc.tile([128, Dt, 72], bf16, tag="g1s")
                    nc.scalar.activation(g1s[:], g1[:, :, 0:72], AF.Copy)
                    nc.vector.copy_predicated(g1s[:], pmv, g1[:, :, 128:200])

                    # logits z[p, d, h] = as1[src] + ad1[dst]
                    z = pcs.tile([128, Dt, 8], f32, tag="z")
                    nc.vector.tensor_tensor(
                        out=z[:],
                        in0=g1s[:, :, 64:72],
                        in1=ad1_all[:, 8 * t:8 * (t + 1)].unsqueeze(1)
                            .broadcast_to([128, Dt, 8]),
                        op=AT.add)
                    # leaky relu (slope 0.2) = max(z, 0.2 z)
                    za = pcs.tile([128, Dt, 8], f32, tag="za")
                    nc.vector.scalar_tensor_tensor(
                        out=za[:], in0=z[:], scalar=NEG_SLOPE, in1=z[:],
                        op0=AT.mult, op1=AT.max)
                    p1 = pcs.tile([128, Dt, 8], bf16, tag="p1")
                    nc.scalar.activation(
                        p1[:].rearrange("p d h -> p (d h)"),
                        za[:].rearrange("p d h -> p (d h)"), AF.Exp)
                    den = pcs.tile([128, 8], f32, tag="den")
                    nc.vector.tensor_reduce(
                        den[:], p1[:].rearrange("p d h -> p h d"),
                        axis=AX.X, op=AT.add)
                    rec = pcs.tile([128, 8], f32, tag="rec")
                    nc.vector.reciprocal(rec[:], den[:])

                    # weighted sum over slots
                    prod = pc.tile([128, Dt, 64], bf16, tag="prod")
                    nc.vector.tensor_tensor(
                        out=prod[:].rearrange("p d (h c) -> p d h c", h=8),
                        in0=g1s[:, :, 0:64].rearrange("p d (h c) -> p d h c", h=8),
                        in1=p1[:].unsqueeze(3)
                            .broadcast_to([128, Dt, 8, 8]),
                        op=AT.mult)
                    o1 = pcs.tile([128, 64], f32, tag="o1")
                    nc.vector.tensor_reduce(
                        o1[:], prod[:].rearrange("p d c -> p c d"),
                        axis=AX.X, op=AT.add)
                    nc.vector.tensor_tensor(
                        out=o1[:].rearrange("p (h c) -> p h c", h=8),
                        in0=o1[:].rearrange("p (h c) -> p h c", h=8),
                        in1=rec[:].unsqueeze(2).broadcast_to([128, 8, 8]),
                        op=AT.mult)
                    nc.vector.tensor_add(o1[:], o1[:], bias1[:])

                    # ELU: max(y, exp(min(y,0)) - 1)
                    mn = pcs.tile([128, 64], f32, tag="mn")
                    nc.vector.tensor_scalar_min(mn[:], o1[:], 0.0)
                    ex = pcs.tile([128, 64], f32, tag="ex")
                    nc.scalar.activation(ex[:], mn[:], AF.Exp)
                    x2 = pcs.tile([128, 64], f32, tag="x2")
                    nc.vector.scalar_tensor_tensor(
                        out=x2[:], in0=ex[:], scalar=-1.0, in1=o1[:],
                        op0=AT.add, op1=AT.max)

                    # proj2
                    tp = pcp.tile([64, 128], f32, tag="tp", space="PSUM")
                    nc.tensor.transpose(out=tp[:], in_=x2[:], identity=identity[:])
                    x2T = pcs.tile([64, 128], bf16, tag="x2T")
                    nc.scalar.activation(x2T[:], tp[:], AF.Copy)
                    ps2 = pcp.tile([128, 34], f32, tag="ps2", space="PSUM")
                    nc.tensor.matmul(out=ps2[:], lhsT=x2T[:], rhs=w2bf[:],
                                     start=True, stop=True)
                    hb2 = pcs.tile([128, 128], bf16, tag="hb2")
                    nc.scalar.activation(hb2[:, 0:33], ps2[:, 0:33], AF.Copy)
                    nc.vector.tensor_copy(ad2_all[:, t:t + 1], ps2[:, 33:34])
                    nc.sync.dma_start(shard2[128 * t:128 * (t + 1), :], hb2[:])
            nc.sync.dma_start(shard2[np_pos:np_pos + 2, :], pad2_d.ap()[:])

            nc.gpsimd.collective_compute(
                "AllGather", AT.bypass, replica_groups=rg,
                ins=[shard2.opt()], outs=[table2.opt()])

            # ---------------- phase E: L2 edges + pooling ----------------
            with (
                tc.tile_pool(name="pe", bufs=2) as pe,
                tc.tile_pool(name="pes", bufs=3) as pes,
                tc.tile_pool(name="pep", bufs=1, space="PSUM") as pep,
            ):
                pool_ps = pep.tile([NG, 32], f32, name="pool_ps", space="PSUM")
                for t in range(NT):
                    Dt = D[t]
                    g2 = pe.tile([128, Dt, 256], bf16, tag="g2")
                    for (so, ns, ccol) in tile_chunks[t]:
                        nidx = 128 * ns
                        gi = nc.gpsimd.dma_gather(
                            g2[:, so:so + ns, :], t2pair,
                            idx_sb[:, ccol:ccol + 8 * ns],
                            nidx, nidx, 256, queue_num=gq())
                        gathers.append(gi)
                    pmv = pm_sb[:, off[t]:off[t] + Dt].unsqueeze(2) \
                        .broadcast_to([128, Dt, 33])
                    g2s = pe.tile([128, Dt, 33], bf16, tag="g2s")
                    nc.scalar.activation(g2s[:], g2[:, :, 0:33], AF.Copy)
                    nc.vector.copy_predicated(g2s[:], pmv, g2[:, :, 128:161])

                    z2 = pes.tile([128, Dt], f32, tag="z2")
                    nc.vector.tensor_tensor(
                        out=z2[:], in0=g2s[:, :, 32],
                        in1=ad2_all[:, t:t + 1].to_broadcast([128, Dt]),
                        op=AT.add)
                    za2 = pes.tile([128, Dt], f32, tag="za2")
                    nc.vector.scalar_tensor_tensor(
                        out=za2[:], in0=z2[:], scalar=NEG_SLOPE, in1=z2[:],
                        op0=AT.mult, op1=AT.max)
                    p2 = pes.tile([128, Dt], bf16, tag="p2")
                    nc.scalar.activation(p2[:], za2[:], AF.Exp)
                    den2 = pes.tile([128, 1], f32, tag="den2")
                    nc.vector.tensor_reduce(den2[:], p2[:], axis=AX.X, op=AT.add)
                    rec2 = pes.tile([128, 1], f32, tag="rec2")
                    nc.vector.reciprocal(rec2[:], den2[:])

                    prod2 = pe.tile([128, Dt, 32], bf16, tag="prod2")
                    nc.vector.tensor_tensor(
                        out=prod2[:],
                        in0=g2s[:, :, 0:32],
                        in1=p2[:].unsqueeze(2).broadcast_to([128, Dt, 32]),
                        op=AT.mult)
                    o2 = pes.tile([128, 32], f32, tag="o2")
                    nc.vector.tensor_reduce(
                        o2[:], prod2[:].rearrange("p d c -> p c d"),
                        axis=AX.X, op=AT.add)
                    nc.vector.tensor_scalar(out=o2[:], in0=o2[:],
                                            scalar1=rec2[:], scalar2=None,
                                            op0=AT.mult)
                    nc.vector.tensor_add(o2[:], o2[:], bias2[:])
                    h2b = pes.tile([128, 32], bf16, tag="h2b")
                    nc.scalar.activation(h2b[:], o2[:], AF.Copy)

                    bt = pes.tile([128, NG], bf16, tag="bt")
                    nc.sync.dma_start(bt[:], bmat_d.ap()[t])
                    nc.tensor.matmul(out=pool_ps[:], lhsT=bt[:], rhs=h2b[:],
                                     start=(t == 0), stop=(t == NT - 1))

                # ---------------- phase F: pool + head ----------------
                pooled = pes.tile([NG, 32], f32, name="pooled")
                nc.vector.tensor_copy(pooled[:], pool_ps[:])
                nc.sync.dma_start(pool_in[:], pooled[:])
                nc.gpsimd.collective_compute(
                    "AllReduce", AT.add, replica_groups=rg,
                    ins=[pool_in.opt()], outs=[pool_out.opt()])
                pooled2 = pes.tile([NG, 32], f32, name="pooled2")
                nc.sync.dma_start(pooled2[:], pool_out[:])
                nc.vector.tensor_scalar(out=pooled2[:], in0=pooled2[:],
                                        scalar1=invc[:], scalar2=None,
                                        op0=AT.mult)
                tpf = pep.tile([32, NG], f32, name="tpf", space="PSUM")
                nc.tensor.transpose(out=tpf[:], in_=pooled2[:],
                                    identity=identity[:NG, :NG])
                pooledT = pes.tile([32, NG], f32, name="pooledT")
                nc.vector.tensor_copy(pooledT[:], tpf[:])
                fin = pep.tile([NG, 2], f32, name="fin", space="PSUM")
                nc.tensor.matmul(out=fin[:], lhsT=pooledT[:], rhs=linw[:],
                                 start=True, stop=True)
                res = pes.tile([NG, 2], f32, name="res")
                nc.vector.tensor_tensor(out=res[:], in0=fin[:],
                                        in1=linb[:NG, :], op=AT.add)
                nc.sync.dma_start(out_d.ap()[:], res[:])

    nc.compile()

    # queue <-> DMASW-lane consistency check (tile assigns lanes round-robin
    # over Pool DMA insts; a lane serving two queues breaks completion sems)
    lane_q = {}
    for bi in gathers:
        inst = getattr(bi, "ins", bi)
        proc = getattr(inst, "bass_scheduled_proc", None)
        q = inst.queue_num
        if proc is None:
            continue
        if proc in lane_q:
            assert lane_q[proc] == q, (
                f"DMASW lane {proc} serves queues {lane_q[proc]} and {q}")
        lane_q[proc] = q
    return nc


# --------------------------------------------------------------------------
# Entry point
# --------------------------------------------------------------------------

def kernel(**inputs):
    from concourse.bass_utils import run_bass_kernel_spmd

    in_maps, meta = _host_prep(**inputs)
    nc = _NEFF_CACHE.get(meta)
    if nc is None:
        nc = _build(meta)
        _NEFF_CACHE[meta] = nc
    res = run_bass_kernel_spmd(nc, in_maps, core_ids=list(range(NC)))
    return np.asarray(res.results[0]["out"], np.float32)


# revision 8
# speedup vs baseline: 3.9128x; 1.2007x over previous
"""Trainium2 Bass kernel for a 2-layer GAT + global mean pool + linear head.

Strategy (8 NeuronCores, SPMD):
  - Nodes sorted by in-degree (desc) and dealt round-robin across the 8
    cores; each core owns the destination side of its shard's edges.
  - Per 128-node tile, in-edges live in a padded ELL table [128, D_t] of
    source indices (D_t = max degree in tile; degree sorting keeps padding
    ~2%). Layer tables (projected features + fused attention logits) are
    bf16, replicated across cores with an AllGather.
  - Edge rows are fetched with gpsimd dma_gather using 512-byte tokens,
    each covering a PAIR of table rows (idx = src_row >> 1 fits int16 for
    the full 50176-row table, so no lo/hi window split and no binomial
    padding blowup). A parity-mask predicated copy selects the right half.
    Gathers are issued round-robin on all 4 SWDGE queues (each queue is a
    separate Q7 core pair, ~3.3x descriptor-generation throughput).
  - Pad slots of real rows point at a pair whose attention logit is -1e30
    (alpha 0); slots of node-less tail rows point at an all-zero pair so
    the softmax denominator stays finite without epsilon ops.
  - Attention softmax + weighted aggregation are dense [128, D_t] ops
    split across DVE and ACT; projections + pooling one-hot run on TensorE.
  - Global mean pool partials are AllReduced; every core computes the tiny
    final head; core 0's output is returned.
"""

import numpy as np
import ml_dtypes

NC = 8
NEG_SLOPE = 0.2
BF16 = ml_dtypes.bfloat16

_NEFF_CACHE = {}


# --------------------------------------------------------------------------
# Host-side preprocessing (indexing / sharding only; all FLOPs on device)
# --------------------------------------------------------------------------

def _host_prep(x, edge_index, batch, W1, att_src1, att_dst1, bias1,
               W2, att_src2, att_dst2, bias2, lin_w, lin_b):
    x = np.ascontiguousarray(np.asarray(x, np.float32))
    ei = np.asarray(edge_index).astype(np.int64)
    batch = np.asarray(batch).astype(np.int64)
    N, F = x.shape
    NG = 64

    src = np.concatenate([ei[0], np.arange(N, dtype=np.int64)])
    dst = np.concatenate([ei[1], np.arange(N, dtype=np.int64)])
    deg = np.bincount(dst, minlength=N)

    order = np.argsort(-deg, kind="stable")
    rank = np.empty(N, np.int64)
    rank[order] = np.arange(N)
    core_of = rank % NC
    pos_of = rank // NC

    SH = -(-N // (NC * 128)) * 128                   # shard rows per core
    NT = SH // 128
    NTBL = SH * NC
    NPAIR = NTBL // 2
    # max positions actually used per core
    max_pos = int(pos_of.max())
    assert max_pos + 4 <= SH, "need >=4 spare rows for pad pairs"
    # zero pair (as=0 -> alpha contribution exp(0), h=0) for node-less rows
    zp_pos = SH - 4                                   # core 0 rows zp, zp+1
    ZPAD = zp_pos // 2                                # pair idx (core 0)
    np_pos = SH - 2                                   # -1e30 pair rows
    NPAD = np_pos // 2
    assert zp_pos % 2 == 0 and np_pos % 2 == 0

    G = core_of * SH + pos_of                        # global table row id

    ds = deg[order]
    D = []
    for t in range(NT):
        r0 = NC * 128 * t
        D.append(int(ds[r0]) if r0 < N else 1)       # sorted desc -> max
    D = [max(d, 1) for d in D]
    off = np.zeros(NT + 1, np.int64)
    off[1:] = np.cumsum(D)
    S = int(off[NT])

    # ELL slot assignment (slot = per-dst running index)
    eorder = np.argsort(rank[dst], kind="stable")
    rs = rank[dst][eorder]
    first = np.searchsorted(rs, rs, side="left")
    d_slot = np.arange(len(rs), dtype=np.int64) - first
    srcg = G[src[eorder]]
    wrow = (srcg >> 1).astype(np.int16)
    par = (srcg & 1).astype(np.float32)
    ek = (rs % NC).astype(np.int64)
    epos = rs // NC
    et = epos // 128
    ep = epos % 128
    ecol = off[et] + d_slot

    idx_all = np.full((NC, 128, S), NPAD, np.int16)
    pm_all = np.zeros((NC, 128, S), np.float32)
    idx_all[ek, ep, ecol] = wrow
    pm_all[ek, ep, ecol] = par
    # node-less tail rows -> zero pair (keeps softmax denominator finite)
    for t in range(NT):
        lo = NC * 128 * t
        hi = min(lo + NC * 128, 10**18)
        if lo + NC * 128 > N:                         # tile has unused ranks
            for k in range(NC):
                # positions pos in tile t with rank = pos*NC + k >= N
                p0 = max(0, -(-(N - k) // NC) - t * 128)   # first unused p
                if p0 < 128:
                    idx_all[k, p0:, off[t]:off[t + 1]] = ZPAD
                    pm_all[k, p0:, off[t]:off[t + 1]] = 0.0

    # gather chunk schedule: (tile, slot_off, nslots), <=8 slots (1024 idxs)
    chunks = []
    for t in range(NT):
        s0 = 0
        while s0 < D[t]:
            ns = min(8, D[t] - s0)
            chunks.append((t, s0, ns))
            s0 += ns
    S16 = sum(8 * ns for (_, _, ns) in chunks)

    # wrapped int16 index input: per chunk, i = slot*128 + p lives at
    # [16r + i%16, i//16] (replicated over the 8 Q7 core groups)
    idx16 = np.empty((NC, 128, S16), np.int16)
    cw = 0
    for (t, so, ns) in chunks:
        for k in range(NC):
            v = idx_all[k, :, off[t] + so: off[t] + so + ns]  # [128, ns]
            flat = v.T.reshape(-1)                            # i = slot*128+p
            wr = flat.reshape(8 * ns, 16).T                   # [16, 8*ns]
            idx16[k, :, cw:cw + 8 * ns] = np.tile(wr, (8, 1))
        cw += 8 * ns
    assert cw == S16

    # pooling one-hot (per core, per tile) and counts
    B = np.zeros((NC, SH, NG), np.float32)
    B[core_of, pos_of, batch] = 1.0
    B = B.reshape(NC, NT, 128, NG).astype(BF16)
    counts = np.bincount(batch, minlength=NG).astype(np.float32)
    inv_counts = (1.0 / np.maximum(counts, 1.0)).reshape(NG, 1)

    # per-core x, transposed for TensorE (feature-major), bf16 on host
    xs = np.zeros((NC, SH, F), np.float32)
    xs[core_of, pos_of] = x
    xt = np.ascontiguousarray(xs.transpose(0, 2, 1)).astype(BF16)

    # fold attention vectors into projection weights
    W1 = np.asarray(W1, np.float32)
    W2 = np.asarray(W2, np.float32)
    a_s1 = np.asarray(att_src1, np.float32)           # [8, 8]
    a_d1 = np.asarray(att_dst1, np.float32)
    a_s2 = np.asarray(att_src2, np.float32)           # [1, 32]
    a_d2 = np.asarray(att_dst2, np.float32)
    W1r = W1.reshape(F, 8, 8)
    w_as1 = np.einsum("khc,hc->kh", W1r, a_s1)
    w_ad1 = np.einsum("khc,hc->kh", W1r, a_d1)
    W1p = np.concatenate([W1, w_as1, w_ad1], axis=1)  # [512, 80]
    W2p = np.concatenate([W2, W2 @ a_s2[0][:, None], W2 @ a_d2[0][:, None]],
                         axis=1)                      # [64, 34]

    pad1 = np.zeros((2, 128), np.float32)
    pad1[:, 64:72] = -1e30
    pad2 = np.zeros((2, 64), np.float32)
    pad2[:, 32] = -1e30

    common = {
        "w1p": np.ascontiguousarray(W1p),
        "w2p": np.ascontiguousarray(W2p),
        "bias1r": np.broadcast_to(np.asarray(bias1, np.float32), (128, 64)).copy(),
        "bias2r": np.broadcast_to(np.asarray(bias2, np.float32), (128, 32)).copy(),
        "linw": np.asarray(lin_w, np.float32).reshape(32, 2).copy(),
        "linbr": np.broadcast_to(np.asarray(lin_b, np.float32), (128, 2)).copy(),
        "invc": inv_counts,
        "pad1": pad1.astype(BF16),
        "pad2": pad2.astype(BF16),
    }
    in_maps = []
    for k in range(NC):
        m = dict(common)
        m["xt"] = xt[k]
        m["idx"] = np.ascontiguousarray(idx16[k])
        m["pm"] = np.ascontiguousarray(pm_all[k].astype(np.uint8))
        m["bmat"] = np.ascontiguousarray(B[k])
        in_maps.append(m)

    meta = (N, F, SH, NT, NTBL, S16, tuple(D), NG, tuple(chunks),
            zp_pos, np_pos)
    return in_maps, meta


# --------------------------------------------------------------------------
# Bass kernel builder
# --------------------------------------------------------------------------

def _build(meta):
    import concourse.bass as bass
    import concourse.bacc as bacc
    import concourse.tile as tile
    import concourse.mybir as mybir
    from concourse.masks import make_identity
    from concourse import library_config

    (N, F, SH, NT, NTBL, S16, D, NG, chunks, zp_pos, np_pos) = meta
    NPAIR = NTBL // 2
    off = [0]
    for d in D:
        off.append(off[-1] + d)
    chunk_cols = []
    cw = 0
    for (_, _, ns) in chunks:
        chunk_cols.append(cw)
        cw += 8 * ns
    tile_chunks = {t: [] for t in range(NT)}
    for ci, (t, so, ns) in enumerate(chunks):
        tile_chunks[t].append((so, ns, chunk_cols[ci]))
    FK = F // 128
    f32 = mybir.dt.float32
    bf16 = mybir.dt.bfloat16
    AT = mybir.AluOpType
    AF = mybir.ActivationFunctionType
    AX = mybir.AxisListType

    nc = bacc.Bacc("TRN2", target_bir_lowering=False, debug=False,
                   num_devices=NC, num_swdge_queues=4)

    xt_d = nc.dram_tensor("xt", [F, SH], bf16, kind="ExternalInput")
    idx_d = nc.dram_tensor("idx", [128, S16], mybir.dt.int16, kind="ExternalInput")
    pm_d = nc.dram_tensor("pm", [128, off[-1]], mybir.dt.uint8, kind="ExternalInput")
    bmat_d = nc.dram_tensor("bmat", [NT, 128, NG], bf16, kind="ExternalInput")
    w1p_d = nc.dram_tensor("w1p", [F, 80], f32, kind="ExternalInput")
    w2p_d = nc.dram_tensor("w2p", [64, 34], f32, kind="ExternalInput")
    b1_d = nc.dram_tensor("bias1r", [128, 64], f32, kind="ExternalInput")
    b2_d = nc.dram_tensor("bias2r", [128, 32], f32, kind="ExternalInput")
    linw_d = nc.dram_tensor("linw", [32, 2], f32, kind="ExternalInput")
    linb_d = nc.dram_tensor("linbr", [128, 2], f32, kind="ExternalInput")
    invc_d = nc.dram_tensor("invc", [NG, 1], f32, kind="ExternalInput")
    pad1_d = nc.dram_tensor("pad1", [2, 128], bf16, kind="ExternalInput")
    pad2_d = nc.dram_tensor("pad2", [2, 64], bf16, kind="ExternalInput")
    out_d = nc.dram_tensor("out", [NG, 2], f32, kind="ExternalOutput")

    rg = [list(range(NC))]
    gathers = []          # (BassInstruction, queue) for post-compile check
    qctr = [0]

    def gq():
        q = qctr[0] % 4
        qctr[0] += 1
        return q

    with tile.TileContext(nc) as tc:
        nc.gpsimd.load_library(library_config.mlp)
        with (
            tc.tile_pool(name="const", bufs=1) as cp,
            tc.tile_pool(name="dram", bufs=1, space="DRAM") as dp,
        ):
            # ---------------- constants ----------------
            identity = cp.tile([128, 128], f32, name="identity")
            make_identity(nc, identity[:])

            w1bf = []
            for kk in range(FK):
                wf = cp.tile([128, 80], f32, name=f"w1f{kk}")
                nc.sync.dma_start(wf[:], w1p_d.ap()[128 * kk:128 * (kk + 1), :])
                wb = cp.tile([128, 80], bf16, name=f"w1b{kk}")
                nc.vector.tensor_copy(wb[:], wf[:])
                w1bf.append(wb)
            w2f = cp.tile([64, 34], f32, name="w2f")
            nc.sync.dma_start(w2f[:], w2p_d.ap()[:])
            w2bf = cp.tile([64, 34], bf16, name="w2bf")
            nc.vector.tensor_copy(w2bf[:], w2f[:])

            bias1 = cp.tile([128, 64], f32, name="bias1")
            nc.sync.dma_start(bias1[:], b1_d.ap()[:])
            bias2 = cp.tile([128, 32], f32, name="bias2")
            nc.sync.dma_start(bias2[:], b2_d.ap()[:])
            linw = cp.tile([32, 2], f32, name="linw_t")
            nc.sync.dma_start(linw[:], linw_d.ap()[:])
            linb = cp.tile([128, 2], f32, name="linb_t")
            nc.sync.dma_start(linb[:], linb_d.ap()[:])
            invc = cp.tile([NG, 1], f32, name="invc_t")
            nc.sync.dma_start(invc[:], invc_d.ap()[:])

            ad1_all = cp.tile([128, NT * 8], f32, name="ad1_all")
            ad2_all = cp.tile([128, NT], f32, name="ad2_all")
            idx_sb = cp.tile([128, S16], mybir.dt.int16, name="idx_sb")
            nc.sync.dma_start(idx_sb[:], idx_d.ap()[:])
            pm_sb = cp.tile([128, off[-1]], mybir.dt.uint8, name="pm_sb")
            nc.sync.dma_start(pm_sb[:], pm_d.ap()[:])

            # ---------------- DRAM tables ----------------
            shard1 = dp.tile([SH, 128], bf16, name="shard1")
            table1 = dp.tile([NTBL, 128], bf16, name="table1", addr_space="Shared")
            shard2 = dp.tile([SH, 64], bf16, name="shard2")
            table2 = dp.tile([NTBL, 64], bf16, name="table2", addr_space="Shared")
            pool_in = dp.tile([NG, 32], f32, name="pool_in")
            pool_out = dp.tile([NG, 32], f32, name="pool_out", addr_space="Shared")

            t1pair = table1[:].rearrange("(r a) c -> r (a c)", a=2)
            t2pair = table2[:].rearrange("(r a) c -> r (a c)", a=2)

            # ---------------- phase A: proj1 -> shard1 ----------------
            with (
                tc.tile_pool(name="pa", bufs=2) as pa,
                tc.tile_pool(name="pax", bufs=1) as pax,
                tc.tile_pool(name="pap", bufs=4, space="PSUM") as pap,
            ):
                xball = []
                for kk in range(FK):
                    xb = pax.tile([128, SH], bf16, name=f"xball{kk}")
                    nc.sync.dma_start(xb[:],
                                      xt_d.ap()[128 * kk:128 * (kk + 1), :])
                    xball.append(xb)
                for t in range(NT):
                    ps = pap.tile([128, 80], f32, tag="proj1")
                    for kk in range(FK):
                        nc.tensor.matmul(
                            out=ps[:],
                            lhsT=xball[kk][:, 128 * t:128 * (t + 1)],
                            rhs=w1bf[kk][:],
                            start=(kk == 0), stop=(kk == FK - 1))
                    hb = pa.tile([128, 128], bf16, tag="hb")
                    nc.scalar.activation(hb[:, 0:72], ps[:, 0:72], AF.Copy)
                    nc.vector.tensor_copy(ad1_all[:, 8 * t:8 * (t + 1)],
                                          ps[:, 72:80])
                    nc.sync.dma_start(shard1[128 * t:128 * (t + 1), :], hb[:])
            nc.sync.dma_start(shard1[np_pos:np_pos + 2, :], pad1_d.ap()[:])

            nc.gpsimd.collective_compute(
                "AllGather", AT.bypass, replica_groups=rg,
                ins=[shard1.opt()], outs=[table1.opt()])

            # ---------------- phase C: L1 edges + proj2 -> shard2 ----------
            with (
                tc.tile_pool(name="pcg", bufs=3) as pcg,
                tc.tile_pool(name="pc", bufs=2) as pc,
                tc.tile_pool(name="pcs", bufs=3) as pcs,
                tc.tile_pool(name="pcp", bufs=4, space="PSUM") as pcp,
            ):
                for t in sorted(range(NT), key=lambda tt: D[tt]):
                    Dt = D[t]
                    g1 = pcg.tile([128, Dt, 256], bf16, tag="g1")
                    for (so, ns, ccol) in tile_chunks[t]:
                        nidx = 128 * ns
                        gi = nc.gpsimd.dma_gather(
                            g1[:, so:so + ns, :], t1pair,
                            idx_sb[:, ccol:ccol + 8 * ns],
                            nidx, nidx, 256, queue_num=gq())
                        gathers.append(gi)
                    # parity select: g1s = par ? odd_half : even_half (72 cols)
                    pmv = pm_sb[:, off[t]:off[t] + Dt].unsqueeze(2) \
                        .broadcast_to([128, Dt, 72])
                    g1s = pc.tile([128, Dt, 72], bf16, tag="g1s")
                    nc.scalar.activation(g1s[:], g1[:, :, 0:72], AF.Copy)
                    nc.vector.copy_predicated(g1s[:], pmv, g1[:, :, 128:200])

                    # logits z[p, d, h] = as1[src] + ad1[dst]
                    z = pcs.tile([128, Dt, 8], f32, tag="z")
                    nc.vector.tensor_tensor(
                        out=z[:],
                        in0=g1s[:, :, 64:72],
                        in1=ad1_all[:, 8 * t:8 * (t + 1)].unsqueeze(1)
                            .broadcast_to([128, Dt, 8]),
                        op=AT.add)
                    # leaky relu (slope 0.2) = max(z, 0.2 z)
                    za = pcs.tile([128, Dt, 8], f32, tag="za")
                    nc.vector.scalar_tensor_tensor(
                        out=za[:], in0=z[:], scalar=NEG_SLOPE, in1=z[:],
                        op0=AT.mult, op1=AT.max)
                    p1 = pcs.tile([128, Dt, 8], bf16, tag="p1")
                    nc.scalar.activation(
                        p1[:].rearrange("p d h -> p (d h)"),
                        za[:].rearrange("p d h -> p (d h)"), AF.Exp)
                    den = pcs.tile([128, 8], f32, tag="den")
                    nc.vector.tensor_reduce(
                        den[:], p1[:].rearrange("p d h -> p h d"),
                        axis=AX.X, op=AT.add)
                    rec = pcs.tile([128, 8], f32, tag="rec")
                    nc.vector.reciprocal(rec[:], den[:])

                    # weighted sum over slots
                    prod = pc.tile([128, Dt, 64], bf16, tag="prod")
                    nc.vector.tensor_tensor(
                        out=prod[:].rearrange("p d (h c) -> p d h c", h=8),
                        in0=g1s[:, :, 0:64].rearrange("p d (h c) -> p d h c", h=8),
                        in1=p1[:].unsqueeze(3)
                            .broadcast_to([128, Dt, 8, 8]),
                        op=AT.mult)
                    o1 = pcs.tile([128, 64], f32, tag="o1")
                    nc.vector.tensor_reduce(
                        o1[:], prod[:].rearrange("p d c -> p c d"),
                        axis=AX.X, op=AT.add)
                    nc.vector.tensor_tensor(
                        out=o1[:].rearrange("p (h c) -> p h c", h=8),
                        in0=o1[:].rearrange("p (h c) -> p h c", h=8),
                        in1=rec[:].unsqueeze(2).broadcast_to([128, 8, 8]),
                        op=AT.mult)
                    nc.vector.tensor_add(o1[:], o1[:], bias1[:])

                    # ELU: max(y, exp(min(y,0)) - 1)
                    mn = pcs.tile([128, 64], f32, tag="mn")
                    nc.vector.tensor_scalar_min(mn[:], o1[:], 0.0)
                    ex = pcs.tile([128, 64], f32, tag="ex")
                    nc.scalar.activation(ex[:], mn[:], AF.Exp)
                    x2 = pcs.tile([128, 64], f32, tag="x2")
                    nc.vector.scalar_tensor_tensor(
                        out=x2[:], in0=ex[:], scalar=-1.0, in1=o1[:],
                        op0=AT.add, op1=AT.max)

                    # proj2
                    tp = pcp.tile([64, 128], f32, tag="tp", space="PSUM")
                    nc.tensor.transpose(out=tp[:], in_=x2[:], identity=identity[:])
                    x2T = pcs.tile([64, 128], bf16, tag="x2T")
                    nc.scalar.activation(x2T[:], tp[:], AF.Copy)
                    ps2 = pcp.tile([128, 34], f32, tag="ps2", space="PSUM")
                    nc.tensor.matmul(out=ps2[:], lhsT=x2T[:], rhs=w2bf[:],
                                     start=True, stop=True)
                    hb2 = pcs.tile([128, 64], bf16, tag="hb2")
                    nc.scalar.activation(hb2[:, 0:33], ps2[:, 0:33], AF.Copy)
                    nc.vector.tensor_copy(ad2_all[:, t:t + 1], ps2[:, 33:34])
                    nc.sync.dma_start(shard2[128 * t:128 * (t + 1), :], hb2[:])
            nc.sync.dma_start(shard2[np_pos:np_pos + 2, :], pad2_d.ap()[:])

            nc.gpsimd.collective_compute(
                "AllGather", AT.bypass, replica_groups=rg,
                ins=[shard2.opt()], outs=[table2.opt()])

            # ---------------- phase E: L2 edges + pooling ----------------
            with (
                tc.tile_pool(name="peg", bufs=3) as peg,
                tc.tile_pool(name="pe", bufs=2) as pe,
                tc.tile_pool(name="pes", bufs=3) as pes,
                tc.tile_pool(name="pep", bufs=1, space="PSUM") as pep,
            ):
                pool_ps = pep.tile([NG, 32], f32, name="pool_ps", space="PSUM")
                etiles = sorted(range(NT), key=lambda tt: D[tt])
                for ei_t, t in enumerate(etiles):
                    Dt = D[t]
                    g2 = peg.tile([128, Dt, 128], bf16, tag="g2")
                    for (so, ns, ccol) in tile_chunks[t]:
                        nidx = 128 * ns
                        gi = nc.gpsimd.dma_gather(
                            g2[:, so:so + ns, :], t2pair,
                            idx_sb[:, ccol:ccol + 8 * ns],
                            nidx, nidx, 128, queue_num=gq())
                        gathers.append(gi)
                    pmv = pm_sb[:, off[t]:off[t] + Dt].unsqueeze(2) \
                        .broadcast_to([128, Dt, 33])
                    g2s = pe.tile([128, Dt, 33], bf16, tag="g2s")
                    nc.scalar.activation(g2s[:], g2[:, :, 0:33], AF.Copy)
                    nc.vector.copy_predicated(g2s[:], pmv, g2[:, :, 64:97])

                    z2 = pes.tile([128, Dt], f32, tag="z2")
                    nc.vector.tensor_tensor(
                        out=z2[:], in0=g2s[:, :, 32],
                        in1=ad2_all[:, t:t + 1].to_broadcast([128, Dt]),
                        op=AT.add)
                    za2 = pes.tile([128, Dt], f32, tag="za2")
                    nc.vector.scalar_tensor_tensor(
                        out=za2[:], in0=z2[:], scalar=NEG_SLOPE, in1=z2[:],
                        op0=AT.mult, op1=AT.max)
                    p2 = pes.tile([128, Dt], bf16, tag="p2")
                    nc.scalar.activation(p2[:], za2[:], AF.Exp)
                    den2 = pes.tile([128, 1], f32, tag="den2")
                    nc.vector.tensor_reduce(den2[:], p2[:], axis=AX.X, op=AT.add)
                    rec2 = pes.tile([128, 1], f32, tag="rec2")
                    nc.vector.reciprocal(rec2[:], den2[:])

                    prod2 = pe.tile([128, Dt, 32], bf16, tag="prod2")
                    nc.vector.tensor_tensor(
                        out=prod2[:],
                        in0=g2s[:, :, 0:32],
                        in1=p2[:].unsqueeze(2).broadcast_to([128, Dt, 32]),
                        op=AT.mult)
                    o2 = pes.tile([128, 32], f32, tag="o2")
                    nc.vector.tensor_reduce(
                        o2[:], prod2[:].rearrange("p d c -> p c d"),
                        axis=AX.X, op=AT.add)
                    nc.vector.tensor_scalar(out=o2[:], in0=o2[:],
                                            scalar1=rec2[:], scalar2=None,
                                            op0=AT.mult)
                    nc.vector.tensor_add(o2[:], o2[:], bias2[:])
                    h2b = pes.tile([128, 32], bf16, tag="h2b")
                    nc.scalar.activation(h2b[:], o2[:], AF.Copy)

                    bt = pes.tile([128, NG], bf16, tag="bt")
                    nc.sync.dma_start(bt[:], bmat_d.ap()[t])
                    nc.tensor.matmul(out=pool_ps[:], lhsT=bt[:], rhs=h2b[:],
                                     start=(ei_t == 0), stop=(ei_t == NT - 1))

                # ---------------- phase F: pool + head ----------------
                pooled = pes.tile([NG, 32], f32, name="pooled")
                nc.vector.tensor_copy(pooled[:], pool_ps[:])
                nc.sync.dma_start(pool_in[:], pooled[:])
                nc.gpsimd.collective_compute(
                    "AllReduce", AT.add, replica_groups=rg,
                    ins=[pool_in.opt()], outs=[pool_out.opt()])
                pooled2 = pes.tile([NG, 32], f32, name="pooled2")
                nc.sync.dma_start(pooled2[:], pool_out[:])
                nc.vector.tensor_scalar(out=pooled2[:], in0=pooled2[:],
                                        scalar1=invc[:], scalar2=None,
                                        op0=AT.mult)
                tpf = pep.tile([32, NG], f32, name="tpf", space="PSUM")
                nc.tensor.transpose(out=tpf[:], in_=pooled2[:],
                                    identity=identity[:NG, :NG])
                pooledT = pes.tile([32, NG], f32, name="pooledT")
                nc.vector.tensor_copy(pooledT[:], tpf[:])
                fin = pep.tile([NG, 2], f32, name="fin", space="PSUM")
                nc.tensor.matmul(out=fin[:], lhsT=pooledT[:], rhs=linw[:],
                                 start=True, stop=True)
                res = pes.tile([NG, 2], f32, name="res")
                nc.vector.tensor_tensor(out=res[:], in0=fin[:],
                                        in1=linb[:NG, :], op=AT.add)
                nc.sync.dma_start(out_d.ap()[:], res[:])

    nc.compile()

    # queue <-> DMASW-lane consistency check (tile assigns lanes round-robin
    # over Pool DMA insts; a lane serving two queues breaks completion sems)
    lane_q = {}
    for bi in gathers:
        inst = getattr(bi, "ins", bi)
        proc = getattr(inst, "bass_scheduled_proc", None)
        q = inst.queue_num
        if proc is None:
            continue
        if proc in lane_q:
            assert lane_q[proc] == q, (
                f"DMASW lane {proc} serves queues {lane_q[proc]} and {q}")
        lane_q[proc] = q
    return nc


# --------------------------------------------------------------------------
# Entry point
# --------------------------------------------------------------------------

def kernel(**inputs):
    from concourse.bass_utils import run_bass_kernel_spmd

    in_maps, meta = _host_prep(**inputs)
    nc = _NEFF_CACHE.get(meta)
    if nc is None:
        nc = _build(meta)
        _NEFF_CACHE[meta] = nc
    res = run_bass_kernel_spmd(nc, in_maps, core_ids=list(range(NC)))
    return np.asarray(res.results[0]["out"], np.float32)


# revision 9
# speedup vs baseline: 4.1705x; 1.0659x over previous
"""Trainium2 Bass kernel for a 2-layer GAT + global mean pool + linear head.

Strategy (8 NeuronCores, SPMD):
  - Nodes sorted by in-degree (desc) and dealt round-robin across the 8
    cores; each core owns the destination side of its shard's edges.
  - Per 128-node tile, in-edges live in a padded ELL table [128, D_t] of
    source indices (D_t = max degree in tile; degree sorting keeps padding
    ~2%). Layer tables (projected features + fused attention logits) are
    bf16, replicated across cores with an AllGather.
  - Edge rows are fetched with gpsimd dma_gather using 512-byte tokens,
    each covering a PAIR of table rows (idx = src_row >> 1 fits int16 for
    the full 50176-row table, so no lo/hi window split and no binomial
    padding blowup). A parity-mask predicated copy selects the right half.
    Gathers are issued round-robin on all 4 SWDGE queues (each queue is a
    separate Q7 core pair, ~3.3x descriptor-generation throughput).
  - Pad slots of real rows point at a pair whose attention logit is -1e30
    (alpha 0); slots of node-less tail rows point at an all-zero pair so
    the softmax denominator stays finite without epsilon ops.
  - Attention softmax + weighted aggregation are dense [128, D_t] ops
    split across DVE and ACT; projections + pooling one-hot run on TensorE.
  - Global mean pool partials are AllReduced; every core computes the tiny
    final head; core 0's output is returned.
"""

import numpy as np
import ml_dtypes

NC = 8
NEG_SLOPE = 0.2
BF16 = ml_dtypes.bfloat16

_NEFF_CACHE = {}


# --------------------------------------------------------------------------
# Host-side preprocessing (indexing / sharding only; all FLOPs on device)
# --------------------------------------------------------------------------

def _host_prep(x, edge_index, batch, W1, att_src1, att_dst1, bias1,
               W2, att_src2, att_dst2, bias2, lin_w, lin_b):
    x = np.ascontiguousarray(np.asarray(x, np.float32))
    ei = np.asarray(edge_index).astype(np.int64)
    batch = np.asarray(batch).astype(np.int64)
    N, F = x.shape
    NG = 64

    src = np.concatenate([ei[0], np.arange(N, dtype=np.int64)])
    dst = np.concatenate([ei[1], np.arange(N, dtype=np.int64)])
    deg = np.bincount(dst, minlength=N)

    order = np.argsort(-deg, kind="stable")
    rank = np.empty(N, np.int64)
    rank[order] = np.arange(N)
    core_of = rank % NC
    pos_of = rank // NC

    SH = -(-N // (NC * 128)) * 128                   # shard rows per core
    NT = SH // 128
    NTBL = SH * NC
    NPAIR = NTBL // 2
    # max positions actually used per core
    max_pos = int(pos_of.max())
    assert max_pos + 4 <= SH, "need >=4 spare rows for pad pairs"
    # zero pair (as=0 -> alpha contribution exp(0), h=0) for node-less rows
    zp_pos = SH - 4                                   # core 0 rows zp, zp+1
    ZPAD = zp_pos // 2                                # pair idx (core 0)
    np_pos = SH - 2                                   # -1e30 pair rows
    NPAD = np_pos // 2
    assert zp_pos % 2 == 0 and np_pos % 2 == 0

    G = core_of * SH + pos_of                        # global table row id

    ds = deg[order]
    D = []
    for t in range(NT):
        r0 = NC * 128 * t
        D.append(int(ds[r0]) if r0 < N else 1)       # sorted desc -> max
    D = [max(d, 1) for d in D]
    off = np.zeros(NT + 1, np.int64)
    off[1:] = np.cumsum(D)
    S = int(off[NT])

    # ELL slot assignment (slot = per-dst running index)
    eorder = np.argsort(rank[dst], kind="stable")
    rs = rank[dst][eorder]
    first = np.searchsorted(rs, rs, side="left")
    d_slot = np.arange(len(rs), dtype=np.int64) - first
    srcg = G[src[eorder]]
    wrow = (srcg >> 1).astype(np.int16)
    par = (srcg & 1).astype(np.float32)
    ek = (rs % NC).astype(np.int64)
    epos = rs // NC
    et = epos // 128
    ep = epos % 128
    ecol = off[et] + d_slot

    idx_all = np.full((NC, 128, S), NPAD, np.int16)
    pm_all = np.zeros((NC, 128, S), np.float32)
    idx_all[ek, ep, ecol] = wrow
    pm_all[ek, ep, ecol] = par
    # node-less tail rows -> zero pair (keeps softmax denominator finite)
    for t in range(NT):
        lo = NC * 128 * t
        hi = min(lo + NC * 128, 10**18)
        if lo + NC * 128 > N:                         # tile has unused ranks
            for k in range(NC):
                # positions pos in tile t with rank = pos*NC + k >= N
                p0 = max(0, -(-(N - k) // NC) - t * 128)   # first unused p
                if p0 < 128:
                    idx_all[k, p0:, off[t]:off[t + 1]] = ZPAD
                    pm_all[k, p0:, off[t]:off[t + 1]] = 0.0

    # gather chunk schedule: (tile, slot_off, nslots), <=8 slots (1024 idxs)
    chunks = []
    for t in range(NT):
        s0 = 0
        while s0 < D[t]:
            ns = min(8, D[t] - s0)
            chunks.append((t, s0, ns))
            s0 += ns
    S16 = sum(8 * ns for (_, _, ns) in chunks)

    # wrapped int16 index input: per chunk, i = slot*128 + p lives at
    # [16r + i%16, i//16] (replicated over the 8 Q7 core groups)
    idx16 = np.empty((NC, 128, S16), np.int16)
    cw = 0
    for (t, so, ns) in chunks:
        for k in range(NC):
            v = idx_all[k, :, off[t] + so: off[t] + so + ns]  # [128, ns]
            flat = v.T.reshape(-1)                            # i = slot*128+p
            wr = flat.reshape(8 * ns, 16).T                   # [16, 8*ns]
            idx16[k, :, cw:cw + 8 * ns] = np.tile(wr, (8, 1))
        cw += 8 * ns
    assert cw == S16

    # pooling one-hot (per core, per tile) and counts
    B = np.zeros((NC, SH, NG), np.float32)
    B[core_of, pos_of, batch] = 1.0
    B = B.reshape(NC, NT, 128, NG).astype(BF16)
    counts = np.bincount(batch, minlength=NG).astype(np.float32)
    inv_counts = (1.0 / np.maximum(counts, 1.0)).reshape(NG, 1)

    # per-core x, transposed for TensorE (feature-major), bf16 on host
    xs = np.zeros((NC, SH, F), np.float32)
    xs[core_of, pos_of] = x
    xt = np.ascontiguousarray(xs.transpose(0, 2, 1)).astype(BF16)

    # fold attention vectors into projection weights
    W1 = np.asarray(W1, np.float32)
    W2 = np.asarray(W2, np.float32)
    a_s1 = np.asarray(att_src1, np.float32)           # [8, 8]
    a_d1 = np.asarray(att_dst1, np.float32)
    a_s2 = np.asarray(att_src2, np.float32)           # [1, 32]
    a_d2 = np.asarray(att_dst2, np.float32)
    W1r = W1.reshape(F, 8, 8)
    w_as1 = np.einsum("khc,hc->kh", W1r, a_s1)
    w_ad1 = np.einsum("khc,hc->kh", W1r, a_d1)
    W1p = np.concatenate([W1, w_as1, w_ad1], axis=1)  # [512, 80]
    W2p = np.concatenate([W2, W2 @ a_s2[0][:, None], W2 @ a_d2[0][:, None]],
                         axis=1)                      # [64, 34]

    pad1 = np.zeros((2, 128), np.float32)
    pad1[:, 64:72] = -1e30
    pad2 = np.zeros((2, 64), np.float32)
    pad2[:, 32] = -1e30

    common = {
        "w1p": np.ascontiguousarray(W1p),
        "w2p": np.ascontiguousarray(W2p),
        "bias1r": np.broadcast_to(np.asarray(bias1, np.float32), (128, 64)).copy(),
        "bias2r": np.broadcast_to(np.asarray(bias2, np.float32), (128, 32)).copy(),
        "linw": np.asarray(lin_w, np.float32).reshape(32, 2).copy(),
        "linbr": np.broadcast_to(np.asarray(lin_b, np.float32), (128, 2)).copy(),
        "invc": inv_counts,
        "pad1": pad1.astype(BF16),
        "pad2": pad2.astype(BF16),
    }
    in_maps = []
    for k in range(NC):
        m = dict(common)
        m["xt"] = xt[k]
        m["idx"] = np.ascontiguousarray(idx16[k])
        m["pm"] = np.ascontiguousarray(pm_all[k].astype(np.uint8))
        m["bmat"] = np.ascontiguousarray(B[k])
        in_maps.append(m)

    meta = (N, F, SH, NT, NTBL, S16, tuple(D), NG, tuple(chunks),
            zp_pos, np_pos)
    return in_maps, meta


# --------------------------------------------------------------------------
# Bass kernel builder
# --------------------------------------------------------------------------

def _build(meta):
    import concourse.bass as bass
    import concourse.bacc as bacc
    import concourse.tile as tile
    import concourse.mybir as mybir
    from concourse.masks import make_identity
    from concourse import library_config

    (N, F, SH, NT, NTBL, S16, D, NG, chunks, zp_pos, np_pos) = meta
    NPAIR = NTBL // 2
    off = [0]
    for d in D:
        off.append(off[-1] + d)
    chunk_cols = []
    cw = 0
    for (_, _, ns) in chunks:
        chunk_cols.append(cw)
        cw += 8 * ns
    tile_chunks = {t: [] for t in range(NT)}
    for ci, (t, so, ns) in enumerate(chunks):
        tile_chunks[t].append((so, ns, chunk_cols[ci]))
    FK = F // 128
    f32 = mybir.dt.float32
    bf16 = mybir.dt.bfloat16
    AT = mybir.AluOpType
    AF = mybir.ActivationFunctionType
    AX = mybir.AxisListType

    nc = bacc.Bacc("TRN2", target_bir_lowering=False, debug=False,
                   num_devices=NC, num_swdge_queues=4)

    xt_d = nc.dram_tensor("xt", [F, SH], bf16, kind="ExternalInput")
    idx_d = nc.dram_tensor("idx", [128, S16], mybir.dt.int16, kind="ExternalInput")
    pm_d = nc.dram_tensor("pm", [128, off[-1]], mybir.dt.uint8, kind="ExternalInput")
    bmat_d = nc.dram_tensor("bmat", [NT, 128, NG], bf16, kind="ExternalInput")
    w1p_d = nc.dram_tensor("w1p", [F, 80], f32, kind="ExternalInput")
    w2p_d = nc.dram_tensor("w2p", [64, 34], f32, kind="ExternalInput")
    b1_d = nc.dram_tensor("bias1r", [128, 64], f32, kind="ExternalInput")
    b2_d = nc.dram_tensor("bias2r", [128, 32], f32, kind="ExternalInput")
    linw_d = nc.dram_tensor("linw", [32, 2], f32, kind="ExternalInput")
    linb_d = nc.dram_tensor("linbr", [128, 2], f32, kind="ExternalInput")
    invc_d = nc.dram_tensor("invc", [NG, 1], f32, kind="ExternalInput")
    pad1_d = nc.dram_tensor("pad1", [2, 128], bf16, kind="ExternalInput")
    pad2_d = nc.dram_tensor("pad2", [2, 64], bf16, kind="ExternalInput")
    out_d = nc.dram_tensor("out", [NG, 2], f32, kind="ExternalOutput")

    rg = [list(range(NC))]
    gathers = []          # (BassInstruction, queue) for post-compile check
    qctr = [0]

    def gq():
        q = qctr[0] % 4
        qctr[0] += 1
        return q

    with tile.TileContext(nc) as tc:
        nc.gpsimd.load_library(library_config.mlp)
        with (
            tc.tile_pool(name="const", bufs=1) as cp,
            tc.tile_pool(name="dram", bufs=1, space="DRAM") as dp,
        ):
            # ---------------- constants ----------------
            identity = cp.tile([128, 128], f32, name="identity")
            make_identity(nc, identity[:])

            w1bf = []
            for kk in range(FK):
                wf = cp.tile([128, 80], f32, name=f"w1f{kk}")
                nc.sync.dma_start(wf[:], w1p_d.ap()[128 * kk:128 * (kk + 1), :])
                wb = cp.tile([128, 80], bf16, name=f"w1b{kk}")
                nc.vector.tensor_copy(wb[:], wf[:])
                w1bf.append(wb)
            w2f = cp.tile([64, 34], f32, name="w2f")
            nc.sync.dma_start(w2f[:], w2p_d.ap()[:])
            w2bf = cp.tile([64, 34], bf16, name="w2bf")
            nc.vector.tensor_copy(w2bf[:], w2f[:])

            bias1 = cp.tile([128, 64], f32, name="bias1")
            nc.sync.dma_start(bias1[:], b1_d.ap()[:])
            bias2 = cp.tile([128, 32], f32, name="bias2")
            nc.sync.dma_start(bias2[:], b2_d.ap()[:])
            linw = cp.tile([32, 2], f32, name="linw_t")
            nc.sync.dma_start(linw[:], linw_d.ap()[:])
            linb = cp.tile([128, 2], f32, name="linb_t")
            nc.sync.dma_start(linb[:], linb_d.ap()[:])
            invc = cp.tile([NG, 1], f32, name="invc_t")
            nc.sync.dma_start(invc[:], invc_d.ap()[:])

            ad1_all = cp.tile([128, NT * 8], f32, name="ad1_all")
            ad2_all = cp.tile([128, NT], f32, name="ad2_all")
            idx_sb = cp.tile([128, S16], mybir.dt.int16, name="idx_sb")
            nc.sync.dma_start(idx_sb[:], idx_d.ap()[:])
            pm_sb = cp.tile([128, off[-1]], mybir.dt.uint8, name="pm_sb")
            nc.sync.dma_start(pm_sb[:], pm_d.ap()[:])

            # ---------------- DRAM tables ----------------
            shard1 = dp.tile([SH, 128], bf16, name="shard1")
            table1 = dp.tile([NTBL, 128], bf16, name="table1", addr_space="Shared")
            shard2 = dp.tile([SH, 64], bf16, name="shard2")
            table2 = dp.tile([NTBL, 64], bf16, name="table2", addr_space="Shared")
            pool_in = dp.tile([NG, 32], f32, name="pool_in")
            pool_out = dp.tile([NG, 32], f32, name="pool_out", addr_space="Shared")

            t1pair = table1[:].rearrange("(r a) c -> r (a c)", a=2)
            t2pair = table2[:].rearrange("(r a) c -> r (a c)", a=2)

            # ---------------- phase A: proj1 -> shard1 ----------------
            with (
                tc.tile_pool(name="pa", bufs=2) as pa,
                tc.tile_pool(name="pax", bufs=1) as pax,
                tc.tile_pool(name="pap", bufs=4, space="PSUM") as pap,
            ):
                xball = []
                for kk in range(FK):
                    xb = pax.tile([128, SH], bf16, name=f"xball{kk}")
                    xball.append(xb)
                CG = 1024
                for c0 in range(0, SH, CG):
                    c1 = min(c0 + CG, SH)
                    for kk in range(FK):
                        nc.sync.dma_start(
                            xball[kk][:, c0:c1],
                            xt_d.ap()[128 * kk:128 * (kk + 1), c0:c1])
                for t in range(NT):
                    ps = pap.tile([128, 80], f32, tag="proj1")
                    for kk in range(FK):
                        nc.tensor.matmul(
                            out=ps[:],
                            lhsT=xball[kk][:, 128 * t:128 * (t + 1)],
                            rhs=w1bf[kk][:],
                            start=(kk == 0), stop=(kk == FK - 1))
                    hb = pa.tile([128, 128], bf16, tag="hb")
                    nc.scalar.activation(hb[:, 0:72], ps[:, 0:72], AF.Copy)
                    nc.vector.tensor_copy(ad1_all[:, 8 * t:8 * (t + 1)],
                                          ps[:, 72:80])
                    nc.sync.dma_start(shard1[128 * t:128 * (t + 1), :], hb[:])
            nc.sync.dma_start(shard1[np_pos:np_pos + 2, :], pad1_d.ap()[:])

            nc.gpsimd.collective_compute(
                "AllGather", AT.bypass, replica_groups=rg,
                ins=[shard1.opt()], outs=[table1.opt()])

            # ---------------- phase C: L1 edges + proj2 -> shard2 ----------
            with (
                tc.tile_pool(name="pcg", bufs=3) as pcg,
                tc.tile_pool(name="pc", bufs=2) as pc,
                tc.tile_pool(name="pcs", bufs=3) as pcs,
                tc.tile_pool(name="pcp", bufs=4, space="PSUM") as pcp,
            ):
                asc = sorted(range(NT), key=lambda tt: D[tt])
                for t in asc[0::2] + asc[1::2][::-1]:
                    Dt = D[t]
                    g1 = pcg.tile([128, Dt, 256], bf16, tag="g1")
                    for (so, ns, ccol) in tile_chunks[t]:
                        nidx = 128 * ns
                        gi = nc.gpsimd.dma_gather(
                            g1[:, so:so + ns, :], t1pair,
                            idx_sb[:, ccol:ccol + 8 * ns],
                            nidx, nidx, 256, queue_num=gq())
                        gathers.append(gi)
                    # parity select: g1s = par ? odd_half : even_half (72 cols)
                    pmv = pm_sb[:, off[t]:off[t] + Dt].unsqueeze(2) \
                        .broadcast_to([128, Dt, 72])
                    g1s = pc.tile([128, Dt, 72], bf16, tag="g1s")
                    nc.scalar.activation(g1s[:], g1[:, :, 0:72], AF.Copy)
                    nc.vector.copy_predicated(g1s[:], pmv, g1[:, :, 128:200])

                    # logits z[p, d, h] = as1[src] + ad1[dst]
                    z = pcs.tile([128, Dt, 8], f32, tag="z")
                    nc.vector.tensor_tensor(
                        out=z[:],
                        in0=g1s[:, :, 64:72],
                        in1=ad1_all[:, 8 * t:8 * (t + 1)].unsqueeze(1)
                            .broadcast_to([128, Dt, 8]),
                        op=AT.add)
                    # leaky relu (slope 0.2) = max(z, 0.2 z)
                    za = pcs.tile([128, Dt, 8], f32, tag="za")
                    nc.vector.scalar_tensor_tensor(
                        out=za[:], in0=z[:], scalar=NEG_SLOPE, in1=z[:],
                        op0=AT.mult, op1=AT.max)
                    p1 = pcs.tile([128, Dt, 8], bf16, tag="p1")
                    nc.scalar.activation(
                        p1[:].rearrange("p d h -> p (d h)"),
                        za[:].rearrange("p d h -> p (d h)"), AF.Exp)
                    den = pcs.tile([128, 8], f32, tag="den")
                    nc.vector.tensor_reduce(
                        den[:], p1[:].rearrange("p d h -> p h d"),
                        axis=AX.X, op=AT.add)
                    rec = pcs.tile([128, 8], f32, tag="rec")
                    nc.vector.reciprocal(rec[:], den[:])

                    # weighted sum over slots
                    prod = pc.tile([128, Dt, 64], bf16, tag="prod")
                    nc.vector.tensor_tensor(
                        out=prod[:].rearrange("p d (h c) -> p d h c", h=8),
                        in0=g1s[:, :, 0:64].rearrange("p d (h c) -> p d h c", h=8),
                        in1=p1[:].unsqueeze(3)
                            .broadcast_to([128, Dt, 8, 8]),
                        op=AT.mult)
                    o1 = pcs.tile([128, 64], f32, tag="o1")
                    nc.vector.tensor_reduce(
                        o1[:], prod[:].rearrange("p d c -> p c d"),
                        axis=AX.X, op=AT.add)
                    nc.vector.tensor_tensor(
                        out=o1[:].rearrange("p (h c) -> p h c", h=8),
                        in0=o1[:].rearrange("p (h c) -> p h c", h=8),
                        in1=rec[:].unsqueeze(2).broadcast_to([128, 8, 8]),
                        op=AT.mult)
                    nc.vector.tensor_add(o1[:], o1[:], bias1[:])

                    # ELU: max(y, exp(min(y,0)) - 1)
                    mn = pcs.tile([128, 64], f32, tag="mn")
                    nc.vector.scalar_tensor_tensor(
                        out=mn[:], in0=o1[:], scalar=0.0, in1=o1[:],
                        op0=AT.min, op1=AT.bypass)
                    ex = pcs.tile([128, 64], f32, tag="ex")
                    nc.scalar.activation(ex[:], mn[:], AF.Exp)
                    x2 = pcs.tile([128, 64], f32, tag="x2")
                    nc.vector.scalar_tensor_tensor(
                        out=x2[:], in0=ex[:], scalar=-1.0, in1=o1[:],
                        op0=AT.add, op1=AT.max)

                    # proj2
                    tp = pcp.tile([64, 128], f32, tag="tp", space="PSUM")
                    nc.tensor.transpose(out=tp[:], in_=x2[:], identity=identity[:])
                    x2T = pcs.tile([64, 128], bf16, tag="x2T")
                    nc.scalar.activation(x2T[:], tp[:], AF.Copy)
                    ps2 = pcp.tile([128, 34], f32, tag="ps2", space="PSUM")
                    nc.tensor.matmul(out=ps2[:], lhsT=x2T[:], rhs=w2bf[:],
                                     start=True, stop=True)
                    hb2 = pcs.tile([128, 64], bf16, tag="hb2")
                    nc.scalar.activation(hb2[:, 0:33], ps2[:, 0:33], AF.Copy)
                    nc.vector.tensor_copy(ad2_all[:, t:t + 1], ps2[:, 33:34])
                    nc.sync.dma_start(shard2[128 * t:128 * (t + 1), :], hb2[:])
            nc.sync.dma_start(shard2[np_pos:np_pos + 2, :], pad2_d.ap()[:])

            nc.gpsimd.collective_compute(
                "AllGather", AT.bypass, replica_groups=rg,
                ins=[shard2.opt()], outs=[table2.opt()])

            # ---------------- phase E: L2 edges + pooling ----------------
            with (
                tc.tile_pool(name="peg", bufs=3) as peg,
                tc.tile_pool(name="pe", bufs=2) as pe,
                tc.tile_pool(name="pes", bufs=3) as pes,
                tc.tile_pool(name="pep", bufs=1, space="PSUM") as pep,
            ):
                pool_ps = pep.tile([NG, 32], f32, name="pool_ps", space="PSUM")
                asc = sorted(range(NT), key=lambda tt: D[tt])
                etiles = asc[0::2] + asc[1::2][::-1]
                for ei_t, t in enumerate(etiles):
                    Dt = D[t]
                    g2 = peg.tile([128, Dt, 128], bf16, tag="g2")
                    for (so, ns, ccol) in tile_chunks[t]:
                        nidx = 128 * ns
                        gi = nc.gpsimd.dma_gather(
                            g2[:, so:so + ns, :], t2pair,
                            idx_sb[:, ccol:ccol + 8 * ns],
                            nidx, nidx, 128, queue_num=gq())
                        gathers.append(gi)
                    pmv = pm_sb[:, off[t]:off[t] + Dt].unsqueeze(2) \
                        .broadcast_to([128, Dt, 33])
                    g2s = pe.tile([128, Dt, 33], bf16, tag="g2s")
                    nc.scalar.activation(g2s[:], g2[:, :, 0:33], AF.Copy)
                    nc.vector.copy_predicated(g2s[:], pmv, g2[:, :, 64:97])

                    z2 = pes.tile([128, Dt], f32, tag="z2")
                    nc.vector.tensor_tensor(
                        out=z2[:], in0=g2s[:, :, 32],
                        in1=ad2_all[:, t:t + 1].to_broadcast([128, Dt]),
                        op=AT.add)
                    za2 = pes.tile([128, Dt], f32, tag="za2")
                    nc.vector.scalar_tensor_tensor(
                        out=za2[:], in0=z2[:], scalar=NEG_SLOPE, in1=z2[:],
                        op0=AT.mult, op1=AT.max)
                    p2 = pes.tile([128, Dt], bf16, tag="p2")
                    nc.scalar.activation(p2[:], za2[:], AF.Exp)
                    den2 = pes.tile([128, 1], f32, tag="den2")
                    nc.vector.tensor_reduce(den2[:], p2[:], axis=AX.X, op=AT.add)
                    rec2 = pes.tile([128, 1], f32, tag="rec2")
                    nc.vector.reciprocal(rec2[:], den2[:])

                    prod2 = pe.tile([128, Dt, 32], bf16, tag="prod2")
                    nc.vector.tensor_tensor(
                        out=prod2[:],
                        in0=g2s[:, :, 0:32],
                        in1=p2[:].unsqueeze(2).broadcast_to([128, Dt, 32]),
                        op=AT.mult)
                    o2 = pes.tile([128, 32], f32, tag="o2")
                    nc.vector.tensor_reduce(
                        o2[:], prod2[:].rearrange("p d c -> p c d"),
                        axis=AX.X, op=AT.add)
                    nc.vector.tensor_tensor(
                        out=o2[:], in0=o2[:],
                        in1=rec2[:].to_broadcast([128, 32]), op=AT.mult)
                    nc.vector.tensor_add(o2[:], o2[:], bias2[:])
                    h2b = pes.tile([128, 32], bf16, tag="h2b")
                    nc.scalar.activation(h2b[:], o2[:], AF.Copy)

                    bt = pes.tile([128, NG], bf16, tag="bt")
                    nc.sync.dma_start(bt[:], bmat_d.ap()[t])
                    nc.tensor.matmul(out=pool_ps[:], lhsT=bt[:], rhs=h2b[:],
                                     start=(ei_t == 0), stop=(ei_t == NT - 1))

                # ---------------- phase F: pool + head ----------------
                pooled = pes.tile([NG, 32], f32, name="pooled")
                nc.vector.tensor_copy(pooled[:], pool_ps[:])
                nc.sync.dma_start(pool_in[:], pooled[:])
                nc.gpsimd.collective_compute(
                    "AllReduce", AT.add, replica_groups=rg,
                    ins=[pool_in.opt()], outs=[pool_out.opt()])
                pooled2 = pes.tile([NG, 32], f32, name="pooled2")
                nc.sync.dma_start(pooled2[:], pool_out[:])
                nc.vector.tensor_tensor(
                    out=pooled2[:], in0=pooled2[:],
                    in1=invc[:].to_broadcast([NG, 32]), op=AT.mult)
                tpf = pep.tile([32, NG], f32, name="tpf", space="PSUM")
                nc.tensor.transpose(out=tpf[:], in_=pooled2[:],
                                    identity=identity[:NG, :NG])
                pooledT = pes.tile([32, NG], f32, name="pooledT")
                nc.vector.tensor_copy(pooledT[:], tpf[:])
                fin = pep.tile([NG, 2], f32, name="fin", space="PSUM")
                nc.tensor.matmul(out=fin[:], lhsT=pooledT[:], rhs=linw[:],
                                 start=True, stop=True)
                res = pes.tile([NG, 2], f32, name="res")
                nc.vector.tensor_tensor(out=res[:], in0=fin[:],
                                        in1=linb[:NG, :], op=AT.add)
                nc.sync.dma_start(out_d.ap()[:], res[:])

    nc.compile()

    # queue <-> DMASW-lane consistency check (tile assigns lanes round-robin
    # over Pool DMA insts; a lane serving two queues breaks completion sems)
    lane_q = {}
    for bi in gathers:
        inst = getattr(bi, "ins", bi)
        proc = getattr(inst, "bass_scheduled_proc", None)
        q = inst.queue_num
        if proc is None:
            continue
        if proc in lane_q:
            assert lane_q[proc] == q, (
                f"DMASW lane {proc} serves queues {lane_q[proc]} and {q}")
        lane_q[proc] = q
    return nc


# --------------------------------------------------------------------------
# Entry point
# --------------------------------------------------------------------------

def kernel(**inputs):
    from concourse.bass_utils import run_bass_kernel_spmd

    in_maps, meta = _host_prep(**inputs)
    nc = _NEFF_CACHE.get(meta)
    if nc is None:
        nc = _build(meta)
        _NEFF_CACHE[meta] = nc
    res = run_bass_kernel_spmd(nc, in_maps, core_ids=list(range(NC)))
    return np.asarray(res.results[0]["out"], np.float32)
